# revision 130
# baseline (speedup 1.0000x reference)
"""Trainium2 Bass kernel for nn_Attention_43301860278871.

Full attention layer: fused QK projection + V projection, interleaved RoPE,
causal SDPA, output projection.  B=2, S=2048, D=2048, H=16, HD=128.

Sharding: 8 cores = 2 batches x 4 head-groups (tensor parallel over heads,
data parallel over batch).  Each core computes 4 heads for one batch and a
partial [S, D] output-projection contribution in fp16; the host upcasts and
sums the 4 partials per batch, so no on-device collectives are needed.

Design:
  * fp8e4 (e4m3) DoubleRow matmuls for ALL four GEMMs (Q/K/V projections +
    output projection).  DoubleRow fuses 2 contraction k-tiles per
    instruction at 0.5 cycles/output-row (4x fp16 throughput in the
    instruction cost model).  Accuracy (tolerance 2e-2) is preserved with a
    hi+lo error-compensation split: each operand T is shipped/computed as
    fp8(T) + fp8(T - fp8(T)), and each GEMM runs 3 DoubleRow sweeps
    (hi*hi, lo_x*hi, hi*lo_w), dropping only the lo*lo term (~0.1%).  Net
    GEMM cost is 0.75x fp16 and rel err stays ~2.5e-3.
  * fp8 scaling: weights are pre-scaled by 2**6 on the host so their
    sigma~0.02 distribution clears e4m3's subnormal floor; the unscale is
    folded into the RoPE trig tables (Q/K), the vt evacuation multiply
    (V), and the final output-copy scale (wo).  The attention output is
    pre-scaled by 32 for ITS fp8 split by setting the PV ones-column to
    1/32 (the row-sum normalization then yields attn*32 for free).
  * DMA discipline: every DMACopy occupies the (serial) HWDGE for a fixed
    ~625ns regardless of size, so loads are whole-tensor batched (55 DMAs
    total vs 246 naive).  Chunk-0's loads are EMITTED in exactly the order
    the interleaved Q/K-main-then-corrections sweep schedule consumes
    them; all non-chunk-0-critical loads are emitted after the kouter so
    the latency-critical rope-swap DMAs sit ahead of them in the SP FIFO.
  * Zero DRAM scratch: K^T (channel-major, full S), Q^T (ping-pong, 2
    chunks), and V (token-major, with a 129th 1/32-column) live in SBUF.
  * Softmax row sums come FREE from the PV matmul (transposed PV, column
    128 of the accumulator is the masked row sum); normalization is a
    per-partition reciprocal+multiply on the DVE during evacuation, a
    [128,128] PE transpose returns the tile to [hd, i], and the fp8 hi/lo
    planes for the wo GEMM are peeled off with one DVE copy + one DVE
    subtract per tile (engine split tuned so Act/DVE land ~50/50).
    Deferred evacuations are split: the DVE front half (recip+normalize)
    runs at the end of stage b; only the transpose+copies wait for the
    next head's drain point.
  * RoPE pair-swap = 2 partition-strided SBUF->SBUF DMAs per pass; cos/sin
    combine all-fp16 on DVE; full trig tables resident (scaled by 2**-6).
  * ONE fused pipeline: projection of chunk c+1 fills sdpa(c)'s exp->mask
    ->PV latency; ALL deferred wo GEMMs fill sdpa(3) (the chunk with the
    longest score streams and no projection work left).  Scores run two
    pairs ahead of exp; PV stage-b (i-tiles 2/3) and deferred evacuations
    cover head boundaries; score pairs are primed across chunk boundaries
    after the filler drain (priming before it parks PE on the rope chain).
  * Causal skipping: PV matmuls for fully-masked j-tiles are not emitted,
    and diagonal-chunk score matmuls slice the moving Q operand to the
    un-masked i-range (start=True bank-zeroing + the mask multiply make
    the skipped region exp(0)=1 -> 0).
  * PSUM: 4 score banks (2-pair lookahead) + 2 PV banks (transpose output
    in the spare tail) + 2-bank rotation for projection/wo accumulators;
    chunk-0 uses its own 8-bank pool (4 Q + 4 K accumulators live
    concurrently so the main sweeps can run back-to-back off the first
    DMA arrivals).

Timeline-simulator exec time: 254645 ns/core (vs 316156 ns fp16 kernel,
403842 ns original baseline; 1.24x / 1.59x); HW rel err vs fp32
reference 2.5e-3.
"""
import sys
sys.path.insert(0, '/opt/trn_rl_repo')

import ml_dtypes
import numpy as np

F8 = ml_dtypes.float8_e4m3

import concourse.bass as bass
import concourse.mybir as mybir
from concourse.bass_utils import run_bass_kernel_spmd
from concourse.tile import TileContext

B, S, D, H = 2, 2048, 2048, 16
HD = D // H            # 128
G = 4                  # head-groups (cores per batch)
HPG = H // G           # heads per core = 4
E = HPG * HD           # per-core projection width = 512
ROPE_BASE = 10000.0
DEBUG_DUMPS = False
SCALE = float(HD) ** -0.5

f32 = mybir.dt.float32
f16 = mybir.dt.float16
f8 = mybir.dt.float8e4     # ml_dtypes.float8_e4m3
WSC = 64.0                 # weight pre-scale 2**6 (host); folded out on-chip
ASC = 32.0                 # attn pre-scale: V ones-col = 1/ASC makes the
                           # row-sum normalization produce attn*ASC for free
DR = mybir.MatmulPerfMode.DoubleRow

KT = D // 128          # 16 contraction tiles
TT = S // 128          # 16 token tiles
TC = S // 512          # 4 token chunks
ET = E // 128          # 4 e-tiles (= heads per core)


# ---------------------------------------------------------------------------
# Workarounds for this walrus build: at most ONE sem wait per instruction.
# Tile's scheduler attaches several; hoist the excess onto NoOps injected on
# the same engine immediately before (sequencer executes waits in order, so
# semantics are identical).
# ---------------------------------------------------------------------------

def _patched_drain_and_barrier(self, tick_clock, wait_clock):
    from concourse.vector_clock import ScopedClock
    drain_inst = self.nc.sync.drain()
    wait_clock.add_sem_waits(
        drain_inst.ins, ScopedClock({None: tick_clock.global_clock})
    )
    si = drain_inst.ins.sync_info
    if si is not None and si.on_wait and len(si.on_wait) > 1:
        waits = list(si.on_wait)
        si.on_wait = waits[:1]
        for w in waits[1:]:
            extra = self.nc.sync.drain()
            esi = extra.ins.sync_info
            if esi is None:
                extra.ins.sync_info = mybir.SyncInfo(on_wait=[w], on_update=[])
            else:
                esi.on_wait = [w]

    self.nc.all_engine_barrier()
    assert self.sems is not None
    popped = self.nc._tile_sem_poison_stack.pop()
    assert popped is self._sem_poison
    self.nc.clear_and_free_semaphores(list(self.sems.allocated().values()))
    self.nc.all_engine_barrier()


def _install_tile_patch():
    import concourse.tile as tile_mod
    tile_mod.TileContext._drain_and_barrier = _patched_drain_and_barrier


def _split_waits(nc, max_waits: int = 1):
    for fn in nc.m.functions:
        for bb in fn.blocks:
            out = []
            changed = False
            for inst in list(bb.instructions):
                si = inst.sync_info
                if si is not None and si.on_wait and len(si.on_wait) > max_waits:
                    waits = list(si.on_wait)
                    for w in waits[:-max_waits]:
                        out.append(mybir.InstNoOp(
                            name=nc.get_next_instruction_name(),
                            engine=inst.engine,
                            sync_info=mybir.SyncInfo(on_wait=[w], on_update=[]),
                        ))
                    si.on_wait = waits[-max_waits:]
                    changed = True
                out.append(inst)
            if changed:
                bb.instructions = out


# ---------------------------------------------------------------------------
# Kernel build (one Bass module, SPMD across the 8 cores via input slices)
# ---------------------------------------------------------------------------

def _build_nc():
    _install_tile_patch()
    nc = bass.Bass()

    xTh = nc.dram_tensor("xTh", [128, KT, S], f8, kind="ExternalInput")
    xTl = nc.dram_tensor("xTl", [128, KT, S], f8, kind="ExternalInput")
    wqTh = nc.dram_tensor("wqTh", [128, KT, ET, 128], f8, kind="ExternalInput")
    wqTl = nc.dram_tensor("wqTl", [128, KT, ET, 128], f8, kind="ExternalInput")
    wkTh = nc.dram_tensor("wkTh", [128, KT, ET, 128], f8, kind="ExternalInput")
    wkTl = nc.dram_tensor("wkTl", [128, KT, ET, 128], f8, kind="ExternalInput")
    wvTh = nc.dram_tensor("wvTh", [128, KT, E], f8, kind="ExternalInput")
    wvTl = nc.dram_tensor("wvTl", [128, KT, E], f8, kind="ExternalInput")
    woTh = nc.dram_tensor("woTh", [128, ET, D], f8, kind="ExternalInput")
    woTl = nc.dram_tensor("woTl", [128, ET, D], f8, kind="ExternalInput")
    cosF = nc.dram_tensor("cosF", [128, S], f16, kind="ExternalInput")
    sinF = nc.dram_tensor("sinF", [128, S], f16, kind="ExternalInput")
    ident = nc.dram_tensor("ident", [128, 128], f16, kind="ExternalInput")
    masks = nc.dram_tensor("masks", [128, ET, 512], f16, kind="ExternalInput")
    out = nc.dram_tensor("out", [S, D], f16, kind="ExternalOutput")
    if DEBUG_DUMPS:
        dqt = nc.dram_tensor("dqt", [128, ET, S], f16, kind="ExternalOutput")
        dkt = nc.dram_tensor("dkt", [128, ET, S], f16, kind="ExternalOutput")
        dvt = nc.dram_tensor("dvt", [128, TT, ET, 129], f16, kind="ExternalOutput")
        doT = nc.dram_tensor("doT", [128, ET, 512], f16, kind="ExternalOutput")

    Exp = mybir.ActivationFunctionType.Exp
    mult = mybir.AluOpType.mult
    add = mybir.AluOpType.add
    sub = mybir.AluOpType.subtract
    divide = mybir.AluOpType.divide

    with TileContext(nc) as tc:
        with (
            nc.allow_low_precision(reason="fp16 operands; fp32 PSUM accum"),
            tc.tile_pool(name="res", bufs=1) as res,
            tc.tile_pool(name="wpool", bufs=1) as wpool,
            tc.tile_pool(name="xpool", bufs=2) as xpool,
            tc.tile_pool(name="stgp", bufs=2) as stgp,
            tc.tile_pool(name="stgq", bufs=1) as stgq,
            tc.tile_pool(name="tp", bufs=2) as tp,
            tc.tile_pool(name="ptp", bufs=9) as ptp,
            tc.tile_pool(name="anp", bufs=4) as anp,
            tc.tile_pool(name="smp", bufs=6) as smp,
            tc.tile_pool(name="oTp", bufs=4) as oTp,
            tc.tile_pool(name="ostp", bufs=2) as ostp,
        ):
            # ---- resident tiles ----
            id_sb = res.tile([128, 128], f16, tag="id")
            m_sb = res.tile([128, ET, 512], f16, tag="masks")
            woh_sb = res.tile([128, ET, D], f8, tag="woh")
            wol_sb = res.tile([128, ET, D], f8, tag="wol")
            # Q ping-pong: chunk c's Q is written during sdpa(c-1) and only
            # read during sdpa(c), so two chunk-sized tiles suffice
            qt_t = [res.tile([128, ET, 512], f16, tag=f"qt{i}",
                             name=f"qt{i}") for i in range(2)]
            kt_sb = res.tile([128, ET, S], f16, tag="kt")
            vt_sb = res.tile([128, TT, ET, 129], f16, tag="vt")

            wqh_sb = wpool.tile([128, KT, ET, 128], f8, tag="wqh")
            wql_sb = wpool.tile([128, KT, ET, 128], f8, tag="wql")
            wkh_sb = wpool.tile([128, KT, ET, 128], f8, tag="wkh")
            wkl_sb = wpool.tile([128, KT, ET, 128], f8, tag="wkl")
            wvh_sb = wpool.tile([128, KT, E], f8, tag="wvh")
            wvl_sb = wpool.tile([128, KT, E], f8, tag="wvl")

            # ones column for the PV row-sum trick (scaled: see ASC)
            nc.vector.memset(vt_sb[:, :, :, 128:129], 1.0 / ASC)

            # full trig tables resident (2 DMAs total; HWDGE fixed cost
            # ~625ns/DMA makes per-chunk reloads a net loss)
            cos_sb = res.tile([128, S], f16, tag="cos")
            sin_sb = res.tile([128, S], f16, tag="sin")


            # ---- DMA loads; chunk 0 pairwise k-interleaved so the k-outer
            # projection streams at DMA pace ----
            # hi planes of wq/x first (they alone gate the main-product
            # sweep); lo planes + later-pass weights follow
            xc_t = {}
            xc0h = xpool.tile([128, KT, 512], f8, tag="xch")
            xc0l = xpool.tile([128, KT, 512], f8, tag="xcl")
            xc_t[0] = (xc0h, xc0l)
            # DMA priority order == chunk-0 sweep consumption order:
            # Q/K hi mains first, then lo correction planes, then V, then
            # everything sdpa(0)+ needs, then wo (chunk-1 time)
            for hf in range(4):
                ks = slice(hf * 4, (hf + 1) * 4)
                nc.sync.dma_start(wqh_sb[:, ks], wqTh[:, ks])
                nc.sync.dma_start(xc0h[:, ks], xTh[:, ks, 0:512])
            for hf in range(2):
                ks = slice(hf * 8, (hf + 1) * 8)
                nc.sync.dma_start(wkh_sb[:, ks], wkTh[:, ks])
            for hf in range(2):
                ks = slice(hf * 8, (hf + 1) * 8)
                nc.sync.dma_start(xc0l[:, ks], xTl[:, ks, 0:512])
            nc.sync.dma_start(wql_sb[:], wqTl[:])
            nc.sync.dma_start(cos_sb[:, 0:512], cosF[:, 0:512])
            nc.sync.dma_start(sin_sb[:, 0:512], sinF[:, 0:512])
            nc.sync.dma_start(wkl_sb[:], wkTl[:])
            nc.sync.dma_start(wvh_sb[:], wvTh[:])
            nc.sync.dma_start(wvl_sb[:], wvTl[:])

            def load_late():
                # emitted after the kouter so the rope-swap DMAs (emitted
                # inside it) sit AHEAD of these in the SP HWDGE FIFO
                nc.sync.dma_start(id_sb[:], ident[:])
                nc.sync.dma_start(m_sb[:], masks[:])
                load_xc(1)
                nc.sync.dma_start(cos_sb[:, 512:], cosF[:, 512:])
                nc.sync.dma_start(sin_sb[:, 512:], sinF[:, 512:])
                nc.sync.dma_start(woh_sb[:], woTh[:])
                nc.sync.dma_start(wol_sb[:], woTl[:])

            def load_xc(tcb):
                th = xpool.tile([128, KT, 512], f8, tag="xch")
                tl = xpool.tile([128, KT, 512], f8, tag="xcl")
                xc_t[tcb] = (th, tl)
                ts = slice(tcb * 512, (tcb + 1) * 512)
                nc.sync.dma_start(th[:], xTh[:, :, ts])
                nc.sync.dma_start(tl[:], xTl[:, :, ts])

            # Staged RoPE: each e-tile of a Q/K pass evacuates into a
            # contiguous fp16 staging tile; ONE pair of partition-strided
            # SBUF->SBUF DMAs then does the channel pair-swap for the whole
            # pass (replacing 4 PE permutation matmuls), and the cos/sin
            # combine runs all-fp16 on the DVE.
            def stage_evac(stag, pq, et):
                nc.scalar.copy(stag[:, et, :], pq[:])

            def rope_combine(stag, dst, dts, tcb, name):
                ts = slice(tcb * 512, (tcb + 1) * 512)
                c_t = cos_sb[:, ts]
                s_t = sin_sb[:, ts]
                stagP = stgq.tile([128, ET, 512], f16, tag="stagP",
                                  name=f"sp{name}")
                nc.sync.dma_start(stagP[0::2], stag[1::2])
                nc.sync.dma_start(stagP[1::2], stag[0::2])
                for et in range(ET):
                    t1 = tp.tile([128, 512], f16, tag="t1")
                    nc.vector.tensor_tensor(t1[:], stag[:, et, :], c_t,
                                            mult)
                    t2 = tp.tile([128, 512], f16, tag="t2")
                    nc.vector.tensor_tensor(t2[:], stagP[:, et, :], s_t,
                                            mult)
                    nc.vector.tensor_tensor(dst[:, et, dts], t1[:], t2[:],
                                            add)

            # ---- chunk-0 projection, k-outer with 4 concurrent
            # accumulators (own 5-bank scratch pool, closed before the
            # steady-state pools open) ----
            with tc.tile_pool(name="ps0", bufs=8, space="PSUM") as ps0:
                KP = KT // 2   # DoubleRow k-pairs

                def sweep0(accs, w_t, x_t, first=False, last=False):
                    for kp in range(KP):
                        for e in range(4):
                            nc.tensor.matmul(
                                accs[e][:],
                                w_t[:, 2 * kp:2 * kp + 2, e, :],
                                x_t[:, 2 * kp:2 * kp + 2, :],
                                start=(first and kp == 0),
                                stop=(last and kp == KP - 1),
                                perf_mode=DR)

                # Q and K main sweeps (hi planes only -- the first DMAs to
                # land) run before any correction sweep; corrections follow
                # in DMA arrival order.  Q+K accumulators fill all 8 banks.
                qaccs = [ps0.tile([128, 512], f32, tag="acc",
                                  name=f"p0q{i}") for i in range(4)]
                kaccs = [ps0.tile([128, 512], f32, tag="acc",
                                  name=f"p0k{i}") for i in range(4)]
                sweep0(qaccs, wqh_sb, xc0h, first=True)
                sweep0(kaccs, wkh_sb, xc0h, first=True)
                sweep0(qaccs, wqh_sb, xc0l)
                sweep0(qaccs, wql_sb, xc0h, last=True)
                stq = stgp.tile([128, ET, 512], f16, tag="stag", name="stq0")
                for et in range(ET):
                    stage_evac(stq, qaccs[et], et)
                rope_combine(stq, qt_t[0], slice(0, 512), 0, "q0")
                sweep0(kaccs, wkh_sb, xc0l)
                sweep0(kaccs, wkl_sb, xc0h, last=True)
                stk = stgp.tile([128, ET, 512], f16, tag="stag", name="stk0")
                for et in range(ET):
                    stage_evac(stk, kaccs[et], et)
                rope_combine(stk, kt_sb, slice(0, 512), 0, "k0")
                vaccs = [ps0.tile([128, 512], f32, tag="acc",
                                  name=f"p0v{i}") for i in range(4)]
                for si, (x_t, w_t) in enumerate(
                        ((xc0h, wvh_sb), (xc0l, wvh_sb))):
                    for kp in range(KP):
                        for tt in range(4):
                            nc.tensor.matmul(
                                vaccs[tt][:],
                                x_t[:, 2 * kp:2 * kp + 2,
                                    tt * 128:(tt + 1) * 128],
                                w_t[:, 2 * kp:2 * kp + 2, :],
                                start=(si == 0 and kp == 0), stop=False,
                                perf_mode=DR)
                # final sweep acc-outer: each vacc stops early so its
                # evacuation overlaps the remaining accs' matmuls
                for tt in range(4):
                    for kp in range(KP):
                        nc.tensor.matmul(
                            vaccs[tt][:],
                            xc0h[:, 2 * kp:2 * kp + 2,
                                 tt * 128:(tt + 1) * 128],
                            wvl_sb[:, 2 * kp:2 * kp + 2, :],
                            start=False, stop=(kp == KP - 1),
                            perf_mode=DR)
                    # early tiles on Act, late on DVE: Act must be clear
                    # when sdpa(0)'s first exp arrives right after
                    if tt < 2:
                        nc.scalar.mul(vt_sb[:, tt, :, 0:128], vaccs[tt][:],
                                      1.0 / WSC)
                    else:
                        nc.vector.tensor_scalar_mul(
                            vt_sb[:, tt, :, 0:128], vaccs[tt][:], 1.0 / WSC)
                load_late()

            # ---- steady state: one fused stream.  SDPA chunk c interleaved
            # with projection of chunk c+1 and output projection of chunk
            # c-1, which share a single 3-bank PSUM rotation ----
            with (
                tc.tile_pool(name="scp", bufs=1, space="PSUM") as scp,
                tc.tile_pool(name="pvp", bufs=1, space="PSUM") as pvp,
                tc.tile_pool(name="psA", bufs=2, space="PSUM") as psA,
            ):
                sc = scp.tile([128, 4, 512], f32, tag="sc")      # 4 banks
                # one full bank per concurrently-accumulating PV group: a
                # start=True matmul zeroes its ENTIRE 2KB bank (pending-zero),
                # so groups must never share a bank
                pv_ab = [pvp.tile([128, 256], f32, tag=f"pv{i}",
                                  name=f"pv{i}") for i in range(2)]

                KP = KT // 2

                def proj_units(tcb):
                    """Generator of filler units projecting chunk tcb."""
                    xch, xcl = xc_t[tcb]
                    if tcb + 1 < TC:
                        load_xc(tcb + 1)
                    for wi, (wh_sb, wl_sb, dst, dts) in enumerate(
                            ((wqh_sb, wql_sb, qt_t[tcb % 2], slice(0, 512)),
                             (wkh_sb, wkl_sb, kt_sb,
                              slice(tcb * 512, (tcb + 1) * 512)))):
                        stag = stgp.tile([128, ET, 512], f16, tag="stag",
                                         name=f"st{tcb}{wi}")
                        for et in range(ET):
                            pq = psA.tile([128, 512], f32, tag="acc")
                            n = 0
                            for w_t, x_t in ((wh_sb, xch), (wh_sb, xcl),
                                             (wl_sb, xch)):
                                for kp in range(KP):
                                    nc.tensor.matmul(
                                        pq[:],
                                        w_t[:, 2 * kp:2 * kp + 2, et, :],
                                        x_t[:, 2 * kp:2 * kp + 2, :],
                                        start=(n == 0), stop=(n == 23),
                                        perf_mode=DR)
                                    n += 1
                                    if n % 6 == 0:
                                        yield
                            stage_evac(stag, pq, et)
                            yield
                        rope_combine(stag, dst, dts, tcb, f"{tcb}{wi}")
                        yield
                    for tt in range(4):
                        pv = psA.tile([128, 512], f32, tag="acc")
                        n = 0
                        for x_t, w_t in ((xch, wvh_sb), (xcl, wvh_sb),
                                         (xch, wvl_sb)):
                            for kp in range(KP):
                                nc.tensor.matmul(
                                    pv[:],
                                    x_t[:, 2 * kp:2 * kp + 2,
                                        tt * 128:(tt + 1) * 128],
                                    w_t[:, 2 * kp:2 * kp + 2, :],
                                    start=(n == 0), stop=(n == 23),
                                    perf_mode=DR)
                                n += 1
                                if n % 6 == 0:
                                    yield
                        nc.scalar.mul(
                            vt_sb[:, tcb * 4 + tt, :, 0:128], pv[:],
                            1.0 / WSC)
                        yield

                ost_of = {}
                wo_ctr = [0]
                OSC = 1.0 / (ASC * WSC)

                def wo_units(oT_c, ic, act_mod=2, final=False,
                             tls=(0, 1, 2, 3)):
                    """Generator of filler units: output projection of
                    chunk ic ((tl,dc) groups, fp8 DoubleRow 3-term)."""
                    oTh_c, oTl_c = oT_c
                    for tl in tls:
                        tsl = slice(tl * 128, (tl + 1) * 128)
                        for dc in range(4):
                            g = wo_ctr[0]
                            wo_ctr[0] += 1
                            dsl = slice(dc * 512, (dc + 1) * 512)
                            po = psA.tile([128, 512], f32, tag="acc")
                            n = 0
                            for a_t, w_t in ((oTh_c, woh_sb), (oTl_c, woh_sb),
                                             (oTh_c, wol_sb)):
                                for ep in range(ET // 2):
                                    nc.tensor.matmul(
                                        po[:],
                                        a_t[:, 2 * ep:2 * ep + 2, tsl],
                                        w_t[:, 2 * ep:2 * ep + 2, dsl],
                                        start=(n == 0), stop=(n == 5),
                                        perf_mode=DR)
                                    n += 1
                            key = (id(oTh_c), tl)
                            if key not in ost_of:
                                ost_t = ostp.tile([128, D], f16, tag="ost",
                                                  name=f"ost{ic}_{tl}")
                                ost_of[key] = (ost_t, 4 * ic + tl)
                            ost, ttk = ost_of[key]
                            if g % act_mod == 0:
                                nc.scalar.mul(ost[:, dsl], po[:], OSC)
                            else:
                                nc.vector.tensor_scalar_mul(
                                    ost[:, dsl], po[:], OSC)
                            if final and tl == 3:
                                # kernel tail: per-dc DMAs pipeline the final
                                # writes with the copies (HWDGE is idle here)
                                nc.sync.dma_start(
                                    out[ttk * 128:(ttk + 1) * 128, dsl],
                                    ost[:, dsl])
                            elif dc == 3:
                                # one whole-row DMA per 128-token tile: the
                                # HWDGE fixed cost dwarfs the extra transfer
                                nc.sync.dma_start(
                                    out[ttk * 128:(ttk + 1) * 128, :],
                                    ost[:])
                            yield

                def chain(*gens):
                    for g in gens:
                        yield from g

                evac_pending = []

                def evac_front(job):
                    # rowsum col is (sum p)/ASC, so the normalize multiply
                    # yields attn*ASC -- a good fp8 range for the wo matmul
                    pvx, _oT, _h, _it = job
                    sm = smp.tile([128, 1], f32, tag="sm")
                    nc.vector.reciprocal(sm[:], pvx[:, 128:129])
                    an = anp.tile([128, 128], f16, tag="an")
                    nc.vector.tensor_scalar_mul(
                        an[:], pvx[:, 0:128], sm[:])
                    return an

                def evac_back(an, job):
                    pvx, (oTh_c, oTl_c), h_, it = job
                    tpv = pvx.bitcast(f16)[:, 280:408]
                    nc.tensor.transpose(tpv, an[:], id_sb[:])
                    osl = slice(it * 128, (it + 1) * 128)
                    nc.vector.tensor_copy(oTh_c[:, h_, osl], tpv)
                    nc.vector.tensor_tensor(
                        oTl_c[:, h_, osl], tpv, oTh_c[:, h_, osl], sub)

                def emit_evac(job):
                    evac_back(evac_front(job), job)

                def sdpa_chunk(ic, filler, n_fill_units, primed=False,
                               prime_ic=None):
                    nj = 4 * (ic + 1)
                    npair = nj // 2
                    qt_c = qt_t[ic % 2]
                    oTh_ic = oTp.tile([128, ET, 512], f8, tag="oTh",
                                      name=f"oTh{ic}")
                    oTl_ic = oTp.tile([128, ET, 512], f8, tag="oTl",
                                      name=f"oTl{ic}")
                    oT_ic = (oTh_ic, oTl_ic)
                    total_steps = ET * (2 * npair + 2)
                    state = {"step": 0, "filled": 0}

                    def fill(n_steps=1):
                        state["step"] += n_steps
                        want = (state["step"] * n_fill_units) // total_steps
                        while state["filled"] < want:
                            try:
                                next(filler)
                            except StopIteration:
                                break
                            state["filled"] += 1

                    for h in range(ET):
                        # diagonal pairs last: the scores-ahead pipeline hides
                        # their exp->mask chain behind earlier pairs' work
                        pairs = (list(range(0, 2 * ic))
                                 + list(range(2 * ic, npair)))

                        def emit_scores(idx, hh=None):
                            if hh is None:
                                hh = h
                            p = pairs[idx]
                            slot = idx % 2
                            for half in range(2):
                                jt = 2 * p + half
                                # causal: i-columns below the diagonal tile
                                # are fully masked; start=True bank-zeroes
                                # them, exp(0)=1 is killed by the mask mult
                                lo = max(0, (jt - 4 * ic) * 128)
                                nc.tensor.matmul(
                                    sc[:, 2 * slot + half, lo:],
                                    kt_sb[:, hh, jt * 128:(jt + 1) * 128],
                                    qt_c[:, hh, lo:], start=True, stop=True)

                        def emit_pv(its, idx, pt_x):
                            p = pairs[idx]
                            for half in range(2):
                                jt = 2 * p + half
                                st = (idx == 0 and half == 0)
                                for sl, it in enumerate(its):
                                    git = 4 * ic + it
                                    if jt > git:
                                        # fully-masked tile: contributes 0
                                        continue
                                    nc.tensor.matmul(
                                        pv_ab[sl][:, 0:129],
                                        pt_x[:, half,
                                             it * 128:(it + 1) * 128],
                                        vt_sb[:, jt, h, :],
                                        start=st, stop=(jt == git))

                        if h == 0 and not primed:
                            emit_scores(0)
                            if npair > 1:
                                emit_scores(1)
                            # no prior-chunk prime covered this exp chain:
                            # release extra filler behind the first scores
                            fill(3)
                        # (h>0: previous head's stage-b primed our scores)
                        # the previous head's deferred it2/3 evacs read the
                        # same PV slots stage-a is about to overwrite - they
                        # MUST all be emitted before the first PV below
                        while evac_pending:
                            evac_back(*evac_pending.pop(0))
                        pts = []
                        # stage a: exp + PV of i-tiles 0/1
                        for idx in range(npair):
                            pt_x = ptp.tile([128, 2, 512], f16, tag="pt")
                            pts.append(pt_x)
                            nc.scalar.activation(
                                pt_x[:],
                                sc[:, 2 * (idx % 2):2 * (idx % 2) + 2, :],
                                Exp, scale=SCALE)
                            m = 2 * pairs[idx] - 4 * ic
                            if m >= 0:
                                nc.vector.tensor_tensor(
                                    pt_x[:], pt_x[:], m_sb[:, m:m + 2, :],
                                    mult)
                            if idx + 2 < npair:
                                emit_scores(idx + 2)
                            fill()
                            emit_pv((0, 1), idx, pt_x)
                        emit_evac((pv_ab[0], oT_ic, h, 0))
                        emit_evac((pv_ab[1], oT_ic, h, 1))
                        fill()
                        # stage b: PV of i-tiles 2/3 off the saved pts -
                        # exp-free PE work that covers the evac chains
                        for idx in range(npair):
                            emit_pv((2, 3), idx, pts[idx])
                            if h + 1 < ET:
                                if idx == 0:
                                    emit_scores(0, h + 1)
                                if idx == min(1, npair - 1) and npair > 1:
                                    emit_scores(1, h + 1)
                            fill()
                        fill()
                        # run the DVE front half (recip+normalize) now; only
                        # the transpose+copies wait for the next drain point,
                        # so the boundary transpose finds `an` ready
                        j2 = (pv_ab[0], oT_ic, h, 2)
                        j3 = (pv_ab[1], oT_ic, h, 3)
                        evac_pending.append((evac_front(j2), j2))
                        evac_pending.append((evac_front(j3), j3))
                    if prime_ic is None:
                        # last chunk: nothing downstream hides the deferred
                        # evacs -- emit them now so they overlap the drain
                        while evac_pending:
                            evac_back(*evac_pending.pop(0))
                    # drain any unconsumed filler at chunk end (before the
                    # prime: prime matmuls queued ahead of leftover filler
                    # would stall PE on the next chunk's rope chain)
                    while True:
                        try:
                            next(filler)
                        except StopIteration:
                            break
                    # prime the NEXT chunk's first two score pairs
                    if prime_ic is not None:
                        qt_n = qt_t[prime_ic % 2]
                        for jt in range(4):
                            nc.tensor.matmul(
                                sc[:, jt, :],
                                kt_sb[:, 0, jt * 128:(jt + 1) * 128],
                                qt_n[:, 0, :], start=True, stop=True)
                    return oT_ic

                oT_hist = {}
                oT_hist[0] = sdpa_chunk(0, proj_units(1), 62, prime_ic=1)
                if DEBUG_DUMPS:
                    nc.sync.dma_start(dkt[:], kt_sb[:])
                    nc.sync.dma_start(dvt[:], vt_sb[:])
                    nc.sync.dma_start(doT[:], oT_hist[0][0][:])
                oT_hist[1] = sdpa_chunk(
                    1, proj_units(2), 66, primed=True, prime_ic=2)
                oT_hist[2] = sdpa_chunk(2, proj_units(3), 62,
                                        primed=True, prime_ic=3)
                # all deferred wo work lands in chunk 3: it has the largest
                # sdpa latency chains (8 pairs/head) and no proj filler left
                oT_hist[3] = sdpa_chunk(
                    3, chain(wo_units(oT_hist[0], 0, act_mod=3),
                             wo_units(oT_hist[1], 1, act_mod=10 ** 9),
                             wo_units(oT_hist[2], 2, act_mod=4)), 44,
                    primed=True)
                while evac_pending:
                    evac_back(*evac_pending.pop(0))
                for _ in wo_units(oT_hist[3], 3, act_mod=1, final=True):
                    pass

    _split_waits(nc)
    return nc


_NC = None


def _get_nc():
    global _NC
    if _NC is None:
        _NC = _build_nc()
    return _NC


# ---------------------------------------------------------------------------
# Host-side prep + gather
# ---------------------------------------------------------------------------

def _rope_tables():
    # pre-scaled by 1/WSC: the rope combine folds the 2**6 fp8 weight
    # pre-scale back out of the Q/K projection PSUM for free
    j = np.arange(0, HD, 2, dtype=np.float32) / HD
    inv_freq = (1.0 / (ROPE_BASE ** j)).astype(np.float32)           # [64]
    t = np.arange(S, dtype=np.float32)
    freqs = np.outer(t, inv_freq)                                    # [S, 64]
    cos = np.cos(freqs).astype(np.float32) / WSC                     # [S, 64]
    sin = np.sin(freqs).astype(np.float32) / WSC
    cosF = np.empty((128, S), dtype=np.float32)
    sinF = np.empty((128, S), dtype=np.float32)
    cosF[0::2, :] = cos.T
    cosF[1::2, :] = cos.T
    sinF[0::2, :] = -sin.T
    sinF[1::2, :] = sin.T
    return cosF.astype(np.float16), sinF.astype(np.float16)


def _static_inputs():
    cosF, sinF = _rope_tables()
    ident = np.eye(128, dtype=np.float16)
    masks = np.zeros((128, ET, 512), dtype=np.float16)
    il = np.arange(512)
    for m in range(ET):
        for p in range(128):
            masks[p, m, :] = (il >= 128 * m + p).astype(np.float16)
    return {
        "cosF": cosF, "sinF": sinF,
        "ident": ident, "masks": masks,
    }


def _fp8_split(a):
    """a (float32) -> (hi, lo) float8_e4m3 with hi + lo ~= a."""
    hi = a.astype(F8)
    lo = (a - hi.astype(np.float32)).astype(F8)
    return hi, lo


def _core_inputs(x, wqk, wv, wo, static, b, g):
    xb = np.ascontiguousarray(x[b].T)                                # [D, S]
    xT = np.ascontiguousarray(
        xb.reshape(KT, 128, S).transpose(1, 0, 2)).astype(np.float32)
    xTh, xTl = _fp8_split(xT)

    wq_g = wqk[E * g:E * (g + 1), :]                                 # [E, D]
    wk_g = wqk[D + E * g:D + E * (g + 1), :]
    wv_g = wv[E * g:E * (g + 1), :]
    wqT = np.ascontiguousarray(
        wq_g.T.reshape(KT, 128, ET, 128)
        .transpose(1, 0, 2, 3)).astype(np.float32) * WSC
    wkT = np.ascontiguousarray(
        wk_g.T.reshape(KT, 128, ET, 128)
        .transpose(1, 0, 2, 3)).astype(np.float32) * WSC
    wvT = np.ascontiguousarray(
        wv_g.T.reshape(KT, 128, E).transpose(1, 0, 2)).astype(np.float32) * WSC
    wqTh, wqTl = _fp8_split(wqT)
    wkTh, wkTl = _fp8_split(wkT)
    wvTh, wvTl = _fp8_split(wvT)
    woT = np.ascontiguousarray(
        wo[:, E * g:E * (g + 1)].T.reshape(ET, 128, D)
        .transpose(1, 0, 2)).astype(np.float32) * WSC
    woTh, woTl = _fp8_split(woT)

    m = dict(static)
    m.update({"xTh": xTh, "xTl": xTl,
              "wqTh": wqTh, "wqTl": wqTl,
              "wkTh": wkTh, "wkTl": wkTl,
              "wvTh": wvTh, "wvTl": wvTl,
              "woTh": woTh, "woTl": woTl})
    return m


def kernel(x, wqk, wv, wo):
    x = np.asarray(x, dtype=np.float32)
    wqk = np.asarray(wqk, dtype=np.float32)
    wv = np.asarray(wv, dtype=np.float32)
    wo = np.asarray(wo, dtype=np.float32)

    nc = _get_nc()
    static = _static_inputs()
    in_maps = [
        _core_inputs(x, wqk, wv, wo, static, c // G, c % G) for c in range(8)
    ]
    res = run_bass_kernel_spmd(nc, in_maps, core_ids=list(range(8)))
    out = np.zeros((B, S, D), dtype=np.float32)
    for c in range(8):
        out[c // G] += res.results[c]["out"].astype(np.float32)
    return out



# revision 134
# speedup vs baseline: 1.0052x; 1.0052x over previous
"""Trainium2 Bass kernel for nn_Attention_43301860278871.

Full attention layer: fused QK projection + V projection, interleaved RoPE,
causal SDPA, output projection.  B=2, S=2048, D=2048, H=16, HD=128.

Sharding: 8 cores = 2 batches x 4 head-groups (tensor parallel over heads,
data parallel over batch).  Each core computes 4 heads for one batch and a
partial [S, D] output-projection contribution in fp16; the host upcasts and
sums the 4 partials per batch, so no on-device collectives are needed.

Design:
  * fp8e4 (e4m3) DoubleRow matmuls for ALL four GEMMs (Q/K/V projections +
    output projection).  DoubleRow fuses 2 contraction k-tiles per
    instruction at 0.5 cycles/output-row (4x fp16 throughput in the
    instruction cost model).  Accuracy (tolerance 2e-2) is preserved with a
    hi+lo error-compensation split: each operand T is shipped/computed as
    fp8(T) + fp8(T - fp8(T)), and each GEMM runs 3 DoubleRow sweeps
    (hi*hi, lo_x*hi, hi*lo_w), dropping only the lo*lo term (~0.1%).  Net
    GEMM cost is 0.75x fp16 and rel err stays ~2.5e-3.
  * fp8 scaling: weights are pre-scaled by 2**6 on the host so their
    sigma~0.02 distribution clears e4m3's subnormal floor; the unscale is
    folded into the RoPE trig tables (Q/K), the vt evacuation multiply
    (V), and the final output-copy scale (wo).  The attention output is
    pre-scaled by 32 for ITS fp8 split by setting the PV ones-column to
    1/32 (the row-sum normalization then yields attn*32 for free).
  * DMA discipline: every DMACopy occupies the (serial) HWDGE for a fixed
    ~625ns regardless of size, so loads are whole-tensor batched (55 DMAs
    total vs 246 naive).  Chunk-0's loads are EMITTED in exactly the order
    the interleaved Q/K-main-then-corrections sweep schedule consumes
    them; all non-chunk-0-critical loads are emitted after the kouter so
    the latency-critical rope-swap DMAs sit ahead of them in the SP FIFO.
  * Zero DRAM scratch: K^T (channel-major, full S), Q^T (ping-pong, 2
    chunks), and V (token-major, with a 129th 1/32-column) live in SBUF.
  * Softmax row sums come FREE from the PV matmul (transposed PV, column
    128 of the accumulator is the masked row sum); normalization is a
    per-partition reciprocal+multiply on the DVE during evacuation, a
    [128,128] PE transpose returns the tile to [hd, i], and the fp8 hi/lo
    planes for the wo GEMM are peeled off with one DVE copy + one DVE
    subtract per tile (engine split tuned so Act/DVE land ~50/50).
    Deferred evacuations are split: the DVE front half (recip+normalize)
    runs at the end of stage b; only the transpose+copies wait for the
    next head's drain point.
  * RoPE pair-swap = 2 partition-strided SBUF->SBUF DMAs per pass; cos/sin
    combine all-fp16 on DVE; full trig tables resident (scaled by 2**-6).
  * ONE fused pipeline: projection of chunk c+1 fills sdpa(c)'s exp->mask
    ->PV latency; ALL deferred wo GEMMs fill sdpa(3) (the chunk with the
    longest score streams and no projection work left).  Scores run two
    pairs ahead of exp; PV stage-b (i-tiles 2/3) and deferred evacuations
    cover head boundaries; score pairs are primed across chunk boundaries
    after the filler drain (priming before it parks PE on the rope chain).
  * Causal skipping: PV matmuls for fully-masked j-tiles are not emitted,
    and diagonal-chunk score matmuls slice the moving Q operand to the
    un-masked i-range (start=True bank-zeroing + the mask multiply make
    the skipped region exp(0)=1 -> 0).
  * PSUM: 4 score banks (2-pair lookahead) + 2 PV banks (transpose output
    in the spare tail) + 2-bank rotation for projection/wo accumulators;
    chunk-0 uses its own 8-bank pool (4 Q + 4 K accumulators live
    concurrently so the main sweeps can run back-to-back off the first
    DMA arrivals).

Timeline-simulator exec time: 254645 ns/core (vs 316156 ns fp16 kernel,
403842 ns original baseline; 1.24x / 1.59x); HW rel err vs fp32
reference 2.5e-3.
"""
import sys
sys.path.insert(0, '/opt/trn_rl_repo')

import ml_dtypes
import numpy as np

F8 = ml_dtypes.float8_e4m3

import concourse.bass as bass
import concourse.mybir as mybir
from concourse.bass_utils import run_bass_kernel_spmd
from concourse.tile import TileContext

B, S, D, H = 2, 2048, 2048, 16
HD = D // H            # 128
G = 4                  # head-groups (cores per batch)
HPG = H // G           # heads per core = 4
E = HPG * HD           # per-core projection width = 512
ROPE_BASE = 10000.0
DEBUG_DUMPS = False
SCALE = float(HD) ** -0.5

f32 = mybir.dt.float32
f16 = mybir.dt.float16
f8 = mybir.dt.float8e4     # ml_dtypes.float8_e4m3
WSC = 64.0                 # weight pre-scale 2**6 (host); folded out on-chip
ASC = 32.0                 # attn pre-scale: V ones-col = 1/ASC makes the
                           # row-sum normalization produce attn*ASC for free
DR = mybir.MatmulPerfMode.DoubleRow

KT = D // 128          # 16 contraction tiles
TT = S // 128          # 16 token tiles
TC = S // 512          # 4 token chunks
ET = E // 128          # 4 e-tiles (= heads per core)


# ---------------------------------------------------------------------------
# Workarounds for this walrus build: at most ONE sem wait per instruction.
# Tile's scheduler attaches several; hoist the excess onto NoOps injected on
# the same engine immediately before (sequencer executes waits in order, so
# semantics are identical).
# ---------------------------------------------------------------------------

def _patched_drain_and_barrier(self, tick_clock, wait_clock):
    from concourse.vector_clock import ScopedClock
    drain_inst = self.nc.sync.drain()
    wait_clock.add_sem_waits(
        drain_inst.ins, ScopedClock({None: tick_clock.global_clock})
    )
    si = drain_inst.ins.sync_info
    if si is not None and si.on_wait and len(si.on_wait) > 1:
        waits = list(si.on_wait)
        si.on_wait = waits[:1]
        for w in waits[1:]:
            extra = self.nc.sync.drain()
            esi = extra.ins.sync_info
            if esi is None:
                extra.ins.sync_info = mybir.SyncInfo(on_wait=[w], on_update=[])
            else:
                esi.on_wait = [w]

    self.nc.all_engine_barrier()
    assert self.sems is not None
    popped = self.nc._tile_sem_poison_stack.pop()
    assert popped is self._sem_poison
    self.nc.clear_and_free_semaphores(list(self.sems.allocated().values()))
    self.nc.all_engine_barrier()


def _install_tile_patch():
    import concourse.tile as tile_mod
    tile_mod.TileContext._drain_and_barrier = _patched_drain_and_barrier


def _split_waits(nc, max_waits: int = 1):
    for fn in nc.m.functions:
        for bb in fn.blocks:
            out = []
            changed = False
            for inst in list(bb.instructions):
                si = inst.sync_info
                if si is not None and si.on_wait and len(si.on_wait) > max_waits:
                    waits = list(si.on_wait)
                    for w in waits[:-max_waits]:
                        out.append(mybir.InstNoOp(
                            name=nc.get_next_instruction_name(),
                            engine=inst.engine,
                            sync_info=mybir.SyncInfo(on_wait=[w], on_update=[]),
                        ))
                    si.on_wait = waits[-max_waits:]
                    changed = True
                out.append(inst)
            if changed:
                bb.instructions = out


# ---------------------------------------------------------------------------
# Kernel build (one Bass module, SPMD across the 8 cores via input slices)
# ---------------------------------------------------------------------------

def _build_nc():
    _install_tile_patch()
    nc = bass.Bass()

    xTh = nc.dram_tensor("xTh", [128, KT, S], f8, kind="ExternalInput")
    xTl = nc.dram_tensor("xTl", [128, KT, S], f8, kind="ExternalInput")
    wqTh = nc.dram_tensor("wqTh", [128, KT, ET, 128], f8, kind="ExternalInput")
    wqTl = nc.dram_tensor("wqTl", [128, KT, ET, 128], f8, kind="ExternalInput")
    wkTh = nc.dram_tensor("wkTh", [128, KT, ET, 128], f8, kind="ExternalInput")
    wkTl = nc.dram_tensor("wkTl", [128, KT, ET, 128], f8, kind="ExternalInput")
    wvTh = nc.dram_tensor("wvTh", [128, KT, E], f8, kind="ExternalInput")
    wvTl = nc.dram_tensor("wvTl", [128, KT, E], f8, kind="ExternalInput")
    woTh = nc.dram_tensor("woTh", [128, ET, D], f8, kind="ExternalInput")
    woTl = nc.dram_tensor("woTl", [128, ET, D], f8, kind="ExternalInput")
    cosF = nc.dram_tensor("cosF", [128, S], f16, kind="ExternalInput")
    sinF = nc.dram_tensor("sinF", [128, S], f16, kind="ExternalInput")
    ident = nc.dram_tensor("ident", [128, 128], f16, kind="ExternalInput")
    masks = nc.dram_tensor("masks", [128, ET, 512], f16, kind="ExternalInput")
    out = nc.dram_tensor("out", [S, D], f16, kind="ExternalOutput")
    if DEBUG_DUMPS:
        dqt = nc.dram_tensor("dqt", [128, ET, S], f16, kind="ExternalOutput")
        dkt = nc.dram_tensor("dkt", [128, ET, S], f16, kind="ExternalOutput")
        dvt = nc.dram_tensor("dvt", [128, TT, ET, 129], f16, kind="ExternalOutput")
        doT = nc.dram_tensor("doT", [128, ET, 512], f16, kind="ExternalOutput")

    Exp = mybir.ActivationFunctionType.Exp
    mult = mybir.AluOpType.mult
    add = mybir.AluOpType.add
    sub = mybir.AluOpType.subtract
    divide = mybir.AluOpType.divide

    with TileContext(nc) as tc:
        with (
            nc.allow_low_precision(reason="fp16 operands; fp32 PSUM accum"),
            tc.tile_pool(name="res", bufs=1) as res,
            tc.tile_pool(name="wpool", bufs=1) as wpool,
            tc.tile_pool(name="xpool", bufs=2) as xpool,
            tc.tile_pool(name="stgp", bufs=2) as stgp,
            tc.tile_pool(name="stgq", bufs=1) as stgq,
            tc.tile_pool(name="tp", bufs=2) as tp,
            tc.tile_pool(name="ptp", bufs=9) as ptp,
            tc.tile_pool(name="anp", bufs=4) as anp,
            tc.tile_pool(name="smp", bufs=6) as smp,
            tc.tile_pool(name="oTp", bufs=4) as oTp,
            tc.tile_pool(name="ostp", bufs=2) as ostp,
        ):
            # ---- resident tiles ----
            id_sb = res.tile([128, 128], f16, tag="id")
            m_sb = res.tile([128, ET, 512], f16, tag="masks")
            woh_sb = res.tile([128, ET, D], f8, tag="woh")
            wol_sb = res.tile([128, ET, D], f8, tag="wol")
            # Q ping-pong: chunk c's Q is written during sdpa(c-1) and only
            # read during sdpa(c), so two chunk-sized tiles suffice
            qt_t = [res.tile([128, ET, 512], f16, tag=f"qt{i}",
                             name=f"qt{i}") for i in range(2)]
            kt_sb = res.tile([128, ET, S], f16, tag="kt")
            vt_sb = res.tile([128, TT, ET, 129], f16, tag="vt")

            wqh_sb = wpool.tile([128, KT, ET, 128], f8, tag="wqh")
            wql_sb = wpool.tile([128, KT, ET, 128], f8, tag="wql")
            wkh_sb = wpool.tile([128, KT, ET, 128], f8, tag="wkh")
            wkl_sb = wpool.tile([128, KT, ET, 128], f8, tag="wkl")
            wvh_sb = wpool.tile([128, KT, E], f8, tag="wvh")
            wvl_sb = wpool.tile([128, KT, E], f8, tag="wvl")

            # ones column for the PV row-sum trick (scaled: see ASC)
            nc.vector.memset(vt_sb[:, :, :, 128:129], 1.0 / ASC)

            # full trig tables resident (2 DMAs total; HWDGE fixed cost
            # ~625ns/DMA makes per-chunk reloads a net loss)
            cos_sb = res.tile([128, S], f16, tag="cos")
            sin_sb = res.tile([128, S], f16, tag="sin")


            # ---- DMA loads; chunk 0 pairwise k-interleaved so the k-outer
            # projection streams at DMA pace ----
            # hi planes of wq/x first (they alone gate the main-product
            # sweep); lo planes + later-pass weights follow
            xc_t = {}
            xc0h = xpool.tile([128, KT, 512], f8, tag="xch")
            xc0l = xpool.tile([128, KT, 512], f8, tag="xcl")
            xc_t[0] = (xc0h, xc0l)
            # DMA priority order == chunk-0 sweep consumption order:
            # Q/K hi mains first, then lo correction planes, then V, then
            # everything sdpa(0)+ needs, then wo (chunk-1 time)
            for hf in range(4):
                ks = slice(hf * 4, (hf + 1) * 4)
                nc.sync.dma_start(wqh_sb[:, ks], wqTh[:, ks])
                nc.sync.dma_start(xc0h[:, ks], xTh[:, ks, 0:512])
            for hf in range(2):
                ks = slice(hf * 8, (hf + 1) * 8)
                nc.sync.dma_start(wkh_sb[:, ks], wkTh[:, ks])
            for hf in range(2):
                ks = slice(hf * 8, (hf + 1) * 8)
                nc.sync.dma_start(xc0l[:, ks], xTl[:, ks, 0:512])
            nc.sync.dma_start(wql_sb[:], wqTl[:])
            nc.sync.dma_start(cos_sb[:, 0:512], cosF[:, 0:512])
            nc.sync.dma_start(sin_sb[:, 0:512], sinF[:, 0:512])
            nc.sync.dma_start(wkl_sb[:], wkTl[:])
            nc.sync.dma_start(wvh_sb[:], wvTh[:])
            nc.sync.dma_start(wvl_sb[:], wvTl[:])

            def load_late():
                # emitted after the kouter so the rope-swap DMAs (emitted
                # inside it) sit AHEAD of these in the SP HWDGE FIFO
                nc.sync.dma_start(id_sb[:], ident[:])
                nc.sync.dma_start(m_sb[:], masks[:])
                load_xc(1)
                nc.sync.dma_start(cos_sb[:, 512:], cosF[:, 512:])
                nc.sync.dma_start(sin_sb[:, 512:], sinF[:, 512:])
                nc.sync.dma_start(woh_sb[:], woTh[:])
                nc.sync.dma_start(wol_sb[:], woTl[:])

            def load_xc(tcb):
                th = xpool.tile([128, KT, 512], f8, tag="xch")
                tl = xpool.tile([128, KT, 512], f8, tag="xcl")
                xc_t[tcb] = (th, tl)
                ts = slice(tcb * 512, (tcb + 1) * 512)
                nc.sync.dma_start(th[:], xTh[:, :, ts])
                nc.sync.dma_start(tl[:], xTl[:, :, ts])

            # Staged RoPE: each e-tile of a Q/K pass evacuates into a
            # contiguous fp16 staging tile; ONE pair of partition-strided
            # SBUF->SBUF DMAs then does the channel pair-swap for the whole
            # pass (replacing 4 PE permutation matmuls), and the cos/sin
            # combine runs all-fp16 on the DVE.
            def stage_evac(stag, pq, et):
                nc.scalar.copy(stag[:, et, :], pq[:])

            def rope_combine(stag, dst, dts, tcb, name):
                ts = slice(tcb * 512, (tcb + 1) * 512)
                c_t = cos_sb[:, ts]
                s_t = sin_sb[:, ts]
                stagP = stgq.tile([128, ET, 512], f16, tag="stagP",
                                  name=f"sp{name}")
                nc.sync.dma_start(stagP[0::2], stag[1::2])
                nc.sync.dma_start(stagP[1::2], stag[0::2])
                for et in range(ET):
                    t1 = tp.tile([128, 512], f16, tag="t1")
                    nc.vector.tensor_tensor(t1[:], stag[:, et, :], c_t,
                                            mult)
                    t2 = tp.tile([128, 512], f16, tag="t2")
                    nc.vector.tensor_tensor(t2[:], stagP[:, et, :], s_t,
                                            mult)
                    nc.vector.tensor_tensor(dst[:, et, dts], t1[:], t2[:],
                                            add)

            # ---- chunk-0 projection, k-outer with 4 concurrent
            # accumulators (own 5-bank scratch pool, closed before the
            # steady-state pools open) ----
            with tc.tile_pool(name="ps0", bufs=8, space="PSUM") as ps0:
                KP = KT // 2   # DoubleRow k-pairs

                def sweep0(accs, w_t, x_t, first=False, last=False):
                    for kp in range(KP):
                        for e in range(4):
                            nc.tensor.matmul(
                                accs[e][:],
                                w_t[:, 2 * kp:2 * kp + 2, e, :],
                                x_t[:, 2 * kp:2 * kp + 2, :],
                                start=(first and kp == 0),
                                stop=(last and kp == KP - 1),
                                perf_mode=DR)

                # Q and K main sweeps (hi planes only -- the first DMAs to
                # land) run before any correction sweep; corrections follow
                # in DMA arrival order.  Q+K accumulators fill all 8 banks.
                qaccs = [ps0.tile([128, 512], f32, tag="acc",
                                  name=f"p0q{i}") for i in range(4)]
                kaccs = [ps0.tile([128, 512], f32, tag="acc",
                                  name=f"p0k{i}") for i in range(4)]
                sweep0(qaccs, wqh_sb, xc0h, first=True)
                sweep0(kaccs, wkh_sb, xc0h, first=True)
                sweep0(qaccs, wqh_sb, xc0l)
                sweep0(qaccs, wql_sb, xc0h, last=True)
                stq = stgp.tile([128, ET, 512], f16, tag="stag", name="stq0")
                for et in range(ET):
                    stage_evac(stq, qaccs[et], et)
                rope_combine(stq, qt_t[0], slice(0, 512), 0, "q0")
                sweep0(kaccs, wkh_sb, xc0l)
                sweep0(kaccs, wkl_sb, xc0h, last=True)
                stk = stgp.tile([128, ET, 512], f16, tag="stag", name="stk0")
                for et in range(ET):
                    stage_evac(stk, kaccs[et], et)
                rope_combine(stk, kt_sb, slice(0, 512), 0, "k0")
                vaccs = [ps0.tile([128, 512], f32, tag="acc",
                                  name=f"p0v{i}") for i in range(4)]
                for si, (x_t, w_t) in enumerate(
                        ((xc0h, wvh_sb), (xc0l, wvh_sb))):
                    for kp in range(KP):
                        for tt in range(4):
                            nc.tensor.matmul(
                                vaccs[tt][:],
                                x_t[:, 2 * kp:2 * kp + 2,
                                    tt * 128:(tt + 1) * 128],
                                w_t[:, 2 * kp:2 * kp + 2, :],
                                start=(si == 0 and kp == 0), stop=False,
                                perf_mode=DR)
                # final sweep acc-outer: each vacc stops early so its
                # evacuation overlaps the remaining accs' matmuls
                for tt in range(4):
                    for kp in range(KP):
                        nc.tensor.matmul(
                            vaccs[tt][:],
                            xc0h[:, 2 * kp:2 * kp + 2,
                                 tt * 128:(tt + 1) * 128],
                            wvl_sb[:, 2 * kp:2 * kp + 2, :],
                            start=False, stop=(kp == KP - 1),
                            perf_mode=DR)
                    # early tiles on Act, late on DVE: Act must be clear
                    # when sdpa(0)'s first exp arrives right after
                    if tt < 2:
                        nc.scalar.mul(vt_sb[:, tt, :, 0:128], vaccs[tt][:],
                                      1.0 / WSC)
                    else:
                        nc.vector.tensor_scalar_mul(
                            vt_sb[:, tt, :, 0:128], vaccs[tt][:], 1.0 / WSC)
                load_late()

            # ---- steady state: one fused stream.  SDPA chunk c interleaved
            # with projection of chunk c+1 and output projection of chunk
            # c-1, which share a single 3-bank PSUM rotation ----
            with (
                tc.tile_pool(name="scp", bufs=1, space="PSUM") as scp,
                tc.tile_pool(name="pvp", bufs=1, space="PSUM") as pvp,
                tc.tile_pool(name="psA", bufs=2, space="PSUM") as psA,
            ):
                sc = scp.tile([128, 4, 512], f32, tag="sc")      # 4 banks
                # one full bank per concurrently-accumulating PV group: a
                # start=True matmul zeroes its ENTIRE 2KB bank (pending-zero),
                # so groups must never share a bank
                pv_ab = [pvp.tile([128, 256], f32, tag=f"pv{i}",
                                  name=f"pv{i}") for i in range(2)]

                KP = KT // 2

                def proj_units(tcb):
                    """Generator of filler units projecting chunk tcb."""
                    xch, xcl = xc_t[tcb]
                    if tcb + 1 < TC:
                        load_xc(tcb + 1)
                    for wi, (wh_sb, wl_sb, dst, dts) in enumerate(
                            ((wqh_sb, wql_sb, qt_t[tcb % 2], slice(0, 512)),
                             (wkh_sb, wkl_sb, kt_sb,
                              slice(tcb * 512, (tcb + 1) * 512)))):
                        stag = stgp.tile([128, ET, 512], f16, tag="stag",
                                         name=f"st{tcb}{wi}")
                        for et in range(ET):
                            pq = psA.tile([128, 512], f32, tag="acc")
                            n = 0
                            for w_t, x_t in ((wh_sb, xch), (wh_sb, xcl),
                                             (wl_sb, xch)):
                                for kp in range(KP):
                                    nc.tensor.matmul(
                                        pq[:],
                                        w_t[:, 2 * kp:2 * kp + 2, et, :],
                                        x_t[:, 2 * kp:2 * kp + 2, :],
                                        start=(n == 0), stop=(n == 23),
                                        perf_mode=DR)
                                    n += 1
                                    if n % 6 == 0:
                                        yield
                            stage_evac(stag, pq, et)
                            yield
                        rope_combine(stag, dst, dts, tcb, f"{tcb}{wi}")
                        yield
                    for tt in range(4):
                        pv = psA.tile([128, 512], f32, tag="acc")
                        n = 0
                        for x_t, w_t in ((xch, wvh_sb), (xcl, wvh_sb),
                                         (xch, wvl_sb)):
                            for kp in range(KP):
                                nc.tensor.matmul(
                                    pv[:],
                                    x_t[:, 2 * kp:2 * kp + 2,
                                        tt * 128:(tt + 1) * 128],
                                    w_t[:, 2 * kp:2 * kp + 2, :],
                                    start=(n == 0), stop=(n == 23),
                                    perf_mode=DR)
                                n += 1
                                if n % 6 == 0:
                                    yield
                        nc.scalar.mul(
                            vt_sb[:, tcb * 4 + tt, :, 0:128], pv[:],
                            1.0 / WSC)
                        yield

                ost_of = {}
                wo_ctr = [0]
                OSC = 1.0 / (ASC * WSC)

                def wo_units(oT_c, ic, act_mod=2, final=False,
                             tls=(0, 1, 2, 3), po_ring=None):
                    """Generator of filler units: output projection of
                    chunk ic ((tl,dc) groups, fp8 DoubleRow 3-term)."""
                    oTh_c, oTl_c = oT_c
                    gi = 0
                    for tl in tls:
                        tsl = slice(tl * 128, (tl + 1) * 128)
                        for dc in range(4):
                            g = wo_ctr[0]
                            wo_ctr[0] += 1
                            dsl = slice(dc * 512, (dc + 1) * 512)
                            if po_ring is not None:
                                po = po_ring[gi % len(po_ring)]
                                gi += 1
                            else:
                                po = psA.tile([128, 512], f32, tag="acc")
                            n = 0
                            for a_t, w_t in ((oTh_c, woh_sb), (oTl_c, woh_sb),
                                             (oTh_c, wol_sb)):
                                for ep in range(ET // 2):
                                    nc.tensor.matmul(
                                        po[:],
                                        a_t[:, 2 * ep:2 * ep + 2, tsl],
                                        w_t[:, 2 * ep:2 * ep + 2, dsl],
                                        start=(n == 0), stop=(n == 5),
                                        perf_mode=DR)
                                    n += 1
                            key = (id(oTh_c), tl)
                            if key not in ost_of:
                                ost_t = ostp.tile([128, D], f16, tag="ost",
                                                  name=f"ost{ic}_{tl}")
                                ost_of[key] = (ost_t, 4 * ic + tl)
                            ost, ttk = ost_of[key]
                            if g % act_mod == 0:
                                nc.scalar.mul(ost[:, dsl], po[:], OSC)
                            else:
                                nc.vector.tensor_scalar_mul(
                                    ost[:, dsl], po[:], OSC)
                            if final and tl == 3:
                                # kernel tail: per-dc DMAs pipeline the final
                                # writes with the copies (HWDGE is idle here)
                                nc.sync.dma_start(
                                    out[ttk * 128:(ttk + 1) * 128, dsl],
                                    ost[:, dsl])
                            elif dc == 3:
                                # one whole-row DMA per 128-token tile: the
                                # HWDGE fixed cost dwarfs the extra transfer
                                nc.sync.dma_start(
                                    out[ttk * 128:(ttk + 1) * 128, :],
                                    ost[:])
                            yield

                def chain(*gens):
                    for g in gens:
                        yield from g

                evac_pending = []

                def evac_front(job):
                    # rowsum col is (sum p)/ASC, so the normalize multiply
                    # yields attn*ASC -- a good fp8 range for the wo matmul
                    pvx, _oT, _h, _it = job
                    sm = smp.tile([128, 1], f32, tag="sm")
                    nc.vector.reciprocal(sm[:], pvx[:, 128:129])
                    an = anp.tile([128, 128], f16, tag="an")
                    nc.vector.tensor_scalar_mul(
                        an[:], pvx[:, 0:128], sm[:])
                    return an

                def evac_back(an, job):
                    pvx, (oTh_c, oTl_c), h_, it = job
                    tpv = pvx.bitcast(f16)[:, 280:408]
                    nc.tensor.transpose(tpv, an[:], id_sb[:])
                    osl = slice(it * 128, (it + 1) * 128)
                    nc.vector.tensor_copy(oTh_c[:, h_, osl], tpv)
                    nc.vector.tensor_tensor(
                        oTl_c[:, h_, osl], tpv, oTh_c[:, h_, osl], sub)

                def emit_evac(job):
                    evac_back(evac_front(job), job)

                def sdpa_chunk(ic, filler, n_fill_units, primed=False,
                               prime_ic=None):
                    nj = 4 * (ic + 1)
                    npair = nj // 2
                    qt_c = qt_t[ic % 2]
                    oTh_ic = oTp.tile([128, ET, 512], f8, tag="oTh",
                                      name=f"oTh{ic}")
                    oTl_ic = oTp.tile([128, ET, 512], f8, tag="oTl",
                                      name=f"oTl{ic}")
                    oT_ic = (oTh_ic, oTl_ic)
                    total_steps = ET * (2 * npair + 2)
                    state = {"step": 0, "filled": 0}

                    def fill(n_steps=1):
                        state["step"] += n_steps
                        want = (state["step"] * n_fill_units) // total_steps
                        while state["filled"] < want:
                            try:
                                next(filler)
                            except StopIteration:
                                break
                            state["filled"] += 1

                    for h in range(ET):
                        # diagonal pairs last: the scores-ahead pipeline hides
                        # their exp->mask chain behind earlier pairs' work
                        pairs = (list(range(0, 2 * ic))
                                 + list(range(2 * ic, npair)))

                        def emit_scores(idx, hh=None):
                            if hh is None:
                                hh = h
                            p = pairs[idx]
                            slot = idx % 2
                            for half in range(2):
                                jt = 2 * p + half
                                # causal: i-columns below the diagonal tile
                                # are fully masked; start=True bank-zeroes
                                # them, exp(0)=1 is killed by the mask mult
                                lo = max(0, (jt - 4 * ic) * 128)
                                nc.tensor.matmul(
                                    sc[:, 2 * slot + half, lo:],
                                    kt_sb[:, hh, jt * 128:(jt + 1) * 128],
                                    qt_c[:, hh, lo:], start=True, stop=True)

                        def emit_pv(its, idx, pt_x):
                            p = pairs[idx]
                            for half in range(2):
                                jt = 2 * p + half
                                st = (idx == 0 and half == 0)
                                for sl, it in enumerate(its):
                                    git = 4 * ic + it
                                    if jt > git:
                                        # fully-masked tile: contributes 0
                                        continue
                                    nc.tensor.matmul(
                                        pv_ab[sl][:, 0:129],
                                        pt_x[:, half,
                                             it * 128:(it + 1) * 128],
                                        vt_sb[:, jt, h, :],
                                        start=st, stop=(jt == git))

                        if h == 0 and not primed:
                            emit_scores(0)
                            if npair > 1:
                                emit_scores(1)
                            # no prior-chunk prime covered this exp chain:
                            # release extra filler behind the first scores
                            fill(3)
                        # (h>0: previous head's stage-b primed our scores)
                        # the previous head's deferred it2/3 evacs read the
                        # same PV slots stage-a is about to overwrite - they
                        # MUST all be emitted before the first PV below
                        while evac_pending:
                            evac_back(*evac_pending.pop(0))
                        pts = []
                        # stage a: exp + PV of i-tiles 0/1
                        for idx in range(npair):
                            pt_x = ptp.tile([128, 2, 512], f16, tag="pt")
                            pts.append(pt_x)
                            nc.scalar.activation(
                                pt_x[:],
                                sc[:, 2 * (idx % 2):2 * (idx % 2) + 2, :],
                                Exp, scale=SCALE)
                            m = 2 * pairs[idx] - 4 * ic
                            if m >= 0:
                                nc.vector.tensor_tensor(
                                    pt_x[:], pt_x[:], m_sb[:, m:m + 2, :],
                                    mult)
                            if idx + 2 < npair:
                                emit_scores(idx + 2)
                            fill()
                            emit_pv((0, 1), idx, pt_x)
                        emit_evac((pv_ab[0], oT_ic, h, 0))
                        emit_evac((pv_ab[1], oT_ic, h, 1))
                        fill()
                        # stage b: PV of i-tiles 2/3 off the saved pts -
                        # exp-free PE work that covers the evac chains
                        for idx in range(npair):
                            emit_pv((2, 3), idx, pts[idx])
                            if h + 1 < ET:
                                if idx == 0:
                                    emit_scores(0, h + 1)
                                if idx == min(1, npair - 1) and npair > 1:
                                    emit_scores(1, h + 1)
                            fill()
                        fill()
                        # run the DVE front half (recip+normalize) now; only
                        # the transpose+copies wait for the next drain point,
                        # so the boundary transpose finds `an` ready
                        j2 = (pv_ab[0], oT_ic, h, 2)
                        j3 = (pv_ab[1], oT_ic, h, 3)
                        evac_pending.append((evac_front(j2), j2))
                        evac_pending.append((evac_front(j3), j3))
                    if prime_ic is None:
                        # last chunk: nothing downstream hides the deferred
                        # evacs -- emit them now so they overlap the drain
                        while evac_pending:
                            evac_back(*evac_pending.pop(0))
                    # drain any unconsumed filler at chunk end (before the
                    # prime: prime matmuls queued ahead of leftover filler
                    # would stall PE on the next chunk's rope chain)
                    while True:
                        try:
                            next(filler)
                        except StopIteration:
                            break
                    # prime the NEXT chunk's first two score pairs
                    if prime_ic is not None:
                        qt_n = qt_t[prime_ic % 2]
                        for jt in range(4):
                            nc.tensor.matmul(
                                sc[:, jt, :],
                                kt_sb[:, 0, jt * 128:(jt + 1) * 128],
                                qt_n[:, 0, :], start=True, stop=True)
                    return oT_ic

                oT_hist = {}
                oT_hist[0] = sdpa_chunk(0, proj_units(1), 62, prime_ic=1)
                if DEBUG_DUMPS:
                    nc.sync.dma_start(dkt[:], kt_sb[:])
                    nc.sync.dma_start(dvt[:], vt_sb[:])
                    nc.sync.dma_start(doT[:], oT_hist[0][0][:])
                oT_hist[1] = sdpa_chunk(
                    1, proj_units(2), 66, primed=True, prime_ic=2)
                oT_hist[2] = sdpa_chunk(2, proj_units(3), 62,
                                        primed=True, prime_ic=3)
                # all deferred wo work lands in chunk 3: it has the largest
                # sdpa latency chains (8 pairs/head) and no proj filler left
                oT_hist[3] = sdpa_chunk(
                    3, chain(wo_units(oT_hist[0], 0, act_mod=3),
                             wo_units(oT_hist[1], 1, act_mod=10 ** 9),
                             wo_units(oT_hist[2], 2, act_mod=4)), 44,
                    primed=True)
                while evac_pending:
                    evac_back(*evac_pending.pop(0))
                # 3-deep accumulator rotation for the serial tail: two psA
                # banks plus ONE reclaimed score bank (single slice -> one
                # group at a time on that tile, no intra-tile group serial)
                po4 = scp.tile([128, 4, 512], f32, tag="sc", name="wo_sc")
                ring = [psA.tile([128, 512], f32, tag="acc", name="wo_pa0"),
                        psA.tile([128, 512], f32, tag="acc", name="wo_pa1"),
                        po4[:, 0, :]]
                for _ in wo_units(oT_hist[3], 3, act_mod=1, final=True,
                                  po_ring=ring):
                    pass

    _split_waits(nc)
    return nc


_NC = None


def _get_nc():
    global _NC
    if _NC is None:
        _NC = _build_nc()
    return _NC


# ---------------------------------------------------------------------------
# Host-side prep + gather
# ---------------------------------------------------------------------------

def _rope_tables():
    # pre-scaled by 1/WSC: the rope combine folds the 2**6 fp8 weight
    # pre-scale back out of the Q/K projection PSUM for free
    j = np.arange(0, HD, 2, dtype=np.float32) / HD
    inv_freq = (1.0 / (ROPE_BASE ** j)).astype(np.float32)           # [64]
    t = np.arange(S, dtype=np.float32)
    freqs = np.outer(t, inv_freq)                                    # [S, 64]
    cos = np.cos(freqs).astype(np.float32) / WSC                     # [S, 64]
    sin = np.sin(freqs).astype(np.float32) / WSC
    cosF = np.empty((128, S), dtype=np.float32)
    sinF = np.empty((128, S), dtype=np.float32)
    cosF[0::2, :] = cos.T
    cosF[1::2, :] = cos.T
    sinF[0::2, :] = -sin.T
    sinF[1::2, :] = sin.T
    return cosF.astype(np.float16), sinF.astype(np.float16)


def _static_inputs():
    cosF, sinF = _rope_tables()
    ident = np.eye(128, dtype=np.float16)
    masks = np.zeros((128, ET, 512), dtype=np.float16)
    il = np.arange(512)
    for m in range(ET):
        for p in range(128):
            masks[p, m, :] = (il >= 128 * m + p).astype(np.float16)
    return {
        "cosF": cosF, "sinF": sinF,
        "ident": ident, "masks": masks,
    }


def _fp8_split(a):
    """a (float32) -> (hi, lo) float8_e4m3 with hi + lo ~= a."""
    hi = a.astype(F8)
    lo = (a - hi.astype(np.float32)).astype(F8)
    return hi, lo


def _core_inputs(x, wqk, wv, wo, static, b, g):
    xb = np.ascontiguousarray(x[b].T)                                # [D, S]
    xT = np.ascontiguousarray(
        xb.reshape(KT, 128, S).transpose(1, 0, 2)).astype(np.float32)
    xTh, xTl = _fp8_split(xT)

    wq_g = wqk[E * g:E * (g + 1), :]                                 # [E, D]
    wk_g = wqk[D + E * g:D + E * (g + 1), :]
    wv_g = wv[E * g:E * (g + 1), :]
    wqT = np.ascontiguousarray(
        wq_g.T.reshape(KT, 128, ET, 128)
        .transpose(1, 0, 2, 3)).astype(np.float32) * WSC
    wkT = np.ascontiguousarray(
        wk_g.T.reshape(KT, 128, ET, 128)
        .transpose(1, 0, 2, 3)).astype(np.float32) * WSC
    wvT = np.ascontiguousarray(
        wv_g.T.reshape(KT, 128, E).transpose(1, 0, 2)).astype(np.float32) * WSC
    wqTh, wqTl = _fp8_split(wqT)
    wkTh, wkTl = _fp8_split(wkT)
    wvTh, wvTl = _fp8_split(wvT)
    woT = np.ascontiguousarray(
        wo[:, E * g:E * (g + 1)].T.reshape(ET, 128, D)
        .transpose(1, 0, 2)).astype(np.float32) * WSC
    woTh, woTl = _fp8_split(woT)

    m = dict(static)
    m.update({"xTh": xTh, "xTl": xTl,
              "wqTh": wqTh, "wqTl": wqTl,
              "wkTh": wkTh, "wkTl": wkTl,
              "wvTh": wvTh, "wvTl": wvTl,
              "woTh": woTh, "woTl": woTl})
    return m


def kernel(x, wqk, wv, wo):
    x = np.asarray(x, dtype=np.float32)
    wqk = np.asarray(wqk, dtype=np.float32)
    wv = np.asarray(wv, dtype=np.float32)
    wo = np.asarray(wo, dtype=np.float32)

    nc = _get_nc()
    static = _static_inputs()
    in_maps = [
        _core_inputs(x, wqk, wv, wo, static, c // G, c % G) for c in range(8)
    ]
    res = run_bass_kernel_spmd(nc, in_maps, core_ids=list(range(8)))
    out = np.zeros((B, S, D), dtype=np.float32)
    for c in range(8):
        out[c // G] += res.results[c]["out"].astype(np.float32)
    return out



# revision 136
# speedup vs baseline: 1.0158x; 1.0106x over previous
"""Trainium2 Bass kernel for nn_Attention_43301860278871.

Full attention layer: fused QK projection + V projection, interleaved RoPE,
causal SDPA, output projection.  B=2, S=2048, D=2048, H=16, HD=128.

Sharding: 8 cores = 2 batches x 4 head-groups (tensor parallel over heads,
data parallel over batch).  Each core computes 4 heads for one batch and a
partial [S, D] output-projection contribution in fp16; the host upcasts and
sums the 4 partials per batch, so no on-device collectives are needed.

Design:
  * fp8e4 (e4m3) DoubleRow matmuls for ALL four GEMMs (Q/K/V projections +
    output projection).  DoubleRow fuses 2 contraction k-tiles per
    instruction at 0.5 cycles/output-row (4x fp16 throughput in the
    instruction cost model).  Accuracy (tolerance 2e-2) is preserved with a
    hi+lo error-compensation split: each operand T is shipped/computed as
    fp8(T) + fp8(T - fp8(T)), and each GEMM runs 3 DoubleRow sweeps
    (hi*hi, lo_x*hi, hi*lo_w), dropping only the lo*lo term (~0.1%).  Net
    GEMM cost is 0.75x fp16 and rel err stays ~2.5e-3.
  * fp8 scaling: weights are pre-scaled by 2**6 on the host so their
    sigma~0.02 distribution clears e4m3's subnormal floor; the unscale is
    folded into the RoPE trig tables (Q/K), the vt evacuation multiply
    (V), and the final output-copy scale (wo).  The attention output is
    pre-scaled by 32 for ITS fp8 split by setting the PV ones-column to
    1/32 (the row-sum normalization then yields attn*32 for free).
  * DMA discipline: every DMACopy occupies the (serial) HWDGE for a fixed
    ~625ns regardless of size, so loads are whole-tensor batched (55 DMAs
    total vs 246 naive).  Chunk-0's loads are EMITTED in exactly the order
    the interleaved Q/K-main-then-corrections sweep schedule consumes
    them; all non-chunk-0-critical loads are emitted after the kouter so
    the latency-critical rope-swap DMAs sit ahead of them in the SP FIFO.
  * Zero DRAM scratch: K^T (channel-major, full S), Q^T (ping-pong, 2
    chunks), and V (token-major, with a 129th 1/32-column) live in SBUF.
  * Softmax row sums come FREE from the PV matmul (transposed PV, column
    128 of the accumulator is the masked row sum); normalization is a
    per-partition reciprocal+multiply on the DVE during evacuation, a
    [128,128] PE transpose returns the tile to [hd, i], and the fp8 hi/lo
    planes for the wo GEMM are peeled off with one DVE copy + one DVE
    subtract per tile (engine split tuned so Act/DVE land ~50/50).
    Deferred evacuations are split: the DVE front half (recip+normalize)
    runs at the end of stage b; only the transpose+copies wait for the
    next head's drain point.
  * RoPE pair-swap = 2 partition-strided SBUF->SBUF DMAs per pass; cos/sin
    combine all-fp16 on DVE; full trig tables resident (scaled by 2**-6).
  * ONE fused pipeline: projection of chunk c+1 fills sdpa(c)'s exp->mask
    ->PV latency; ALL deferred wo GEMMs fill sdpa(3) (the chunk with the
    longest score streams and no projection work left).  Scores run two
    pairs ahead of exp; PV stage-b (i-tiles 2/3) and deferred evacuations
    cover head boundaries; score pairs are primed across chunk boundaries
    after the filler drain (priming before it parks PE on the rope chain).
  * Causal skipping: PV matmuls for fully-masked j-tiles are not emitted,
    and diagonal-chunk score matmuls slice the moving Q operand to the
    un-masked i-range (start=True bank-zeroing + the mask multiply make
    the skipped region exp(0)=1 -> 0).
  * PSUM: 4 score banks (2-pair lookahead) + 2 PV banks (transpose output
    in the spare tail) + 2-bank rotation for projection/wo accumulators;
    chunk-0 uses its own 8-bank pool (4 Q + 4 K accumulators live
    concurrently so the main sweeps can run back-to-back off the first
    DMA arrivals).

Timeline-simulator exec time: 253331 ns/core (vs 316156 ns fp16 kernel,
403842 ns original baseline; 1.25x / 1.59x); HW rel err vs fp32
reference 2.5e-3.
"""
import sys
sys.path.insert(0, '/opt/trn_rl_repo')

import ml_dtypes
import numpy as np

F8 = ml_dtypes.float8_e4m3

import concourse.bass as bass
import concourse.mybir as mybir
from concourse.bass_utils import run_bass_kernel_spmd
from concourse.tile import TileContext

B, S, D, H = 2, 2048, 2048, 16
HD = D // H            # 128
G = 4                  # head-groups (cores per batch)
HPG = H // G           # heads per core = 4
E = HPG * HD           # per-core projection width = 512
ROPE_BASE = 10000.0
DEBUG_DUMPS = False
SCALE = float(HD) ** -0.5

f32 = mybir.dt.float32
f16 = mybir.dt.float16
f8 = mybir.dt.float8e4     # ml_dtypes.float8_e4m3
WSC = 64.0                 # weight pre-scale 2**6 (host); folded out on-chip
ASC = 32.0                 # attn pre-scale: V ones-col = 1/ASC makes the
                           # row-sum normalization produce attn*ASC for free
DR = mybir.MatmulPerfMode.DoubleRow

KT = D // 128          # 16 contraction tiles
TT = S // 128          # 16 token tiles
TC = S // 512          # 4 token chunks
ET = E // 128          # 4 e-tiles (= heads per core)


# ---------------------------------------------------------------------------
# Workarounds for this walrus build: at most ONE sem wait per instruction.
# Tile's scheduler attaches several; hoist the excess onto NoOps injected on
# the same engine immediately before (sequencer executes waits in order, so
# semantics are identical).
# ---------------------------------------------------------------------------

def _patched_drain_and_barrier(self, tick_clock, wait_clock):
    from concourse.vector_clock import ScopedClock
    drain_inst = self.nc.sync.drain()
    wait_clock.add_sem_waits(
        drain_inst.ins, ScopedClock({None: tick_clock.global_clock})
    )
    si = drain_inst.ins.sync_info
    if si is not None and si.on_wait and len(si.on_wait) > 1:
        waits = list(si.on_wait)
        si.on_wait = waits[:1]
        for w in waits[1:]:
            extra = self.nc.sync.drain()
            esi = extra.ins.sync_info
            if esi is None:
                extra.ins.sync_info = mybir.SyncInfo(on_wait=[w], on_update=[])
            else:
                esi.on_wait = [w]

    self.nc.all_engine_barrier()
    assert self.sems is not None
    popped = self.nc._tile_sem_poison_stack.pop()
    assert popped is self._sem_poison
    self.nc.clear_and_free_semaphores(list(self.sems.allocated().values()))
    self.nc.all_engine_barrier()


def _install_tile_patch():
    import concourse.tile as tile_mod
    tile_mod.TileContext._drain_and_barrier = _patched_drain_and_barrier


def _split_waits(nc, max_waits: int = 1):
    for fn in nc.m.functions:
        for bb in fn.blocks:
            out = []
            changed = False
            for inst in list(bb.instructions):
                si = inst.sync_info
                if si is not None and si.on_wait and len(si.on_wait) > max_waits:
                    waits = list(si.on_wait)
                    for w in waits[:-max_waits]:
                        out.append(mybir.InstNoOp(
                            name=nc.get_next_instruction_name(),
                            engine=inst.engine,
                            sync_info=mybir.SyncInfo(on_wait=[w], on_update=[]),
                        ))
                    si.on_wait = waits[-max_waits:]
                    changed = True
                out.append(inst)
            if changed:
                bb.instructions = out


# ---------------------------------------------------------------------------
# Kernel build (one Bass module, SPMD across the 8 cores via input slices)
# ---------------------------------------------------------------------------

def _build_nc():
    _install_tile_patch()
    nc = bass.Bass()

    xTh = nc.dram_tensor("xTh", [128, KT, S], f8, kind="ExternalInput")
    xTl = nc.dram_tensor("xTl", [128, KT, S], f8, kind="ExternalInput")
    wqTh = nc.dram_tensor("wqTh", [128, KT, ET, 128], f8, kind="ExternalInput")
    wqTl = nc.dram_tensor("wqTl", [128, KT, ET, 128], f8, kind="ExternalInput")
    wkTh = nc.dram_tensor("wkTh", [128, KT, ET, 128], f8, kind="ExternalInput")
    wkTl = nc.dram_tensor("wkTl", [128, KT, ET, 128], f8, kind="ExternalInput")
    wvTh = nc.dram_tensor("wvTh", [128, KT, E], f8, kind="ExternalInput")
    wvTl = nc.dram_tensor("wvTl", [128, KT, E], f8, kind="ExternalInput")
    woTh = nc.dram_tensor("woTh", [128, ET, D], f8, kind="ExternalInput")
    woTl = nc.dram_tensor("woTl", [128, ET, D], f8, kind="ExternalInput")
    cosF = nc.dram_tensor("cosF", [128, S], f16, kind="ExternalInput")
    sinF = nc.dram_tensor("sinF", [128, S], f16, kind="ExternalInput")
    ident = nc.dram_tensor("ident", [128, 128], f16, kind="ExternalInput")
    masks = nc.dram_tensor("masks", [128, ET, 512], f16, kind="ExternalInput")
    out = nc.dram_tensor("out", [S, D], f16, kind="ExternalOutput")
    if DEBUG_DUMPS:
        dqt = nc.dram_tensor("dqt", [128, ET, S], f16, kind="ExternalOutput")
        dkt = nc.dram_tensor("dkt", [128, ET, S], f16, kind="ExternalOutput")
        dvt = nc.dram_tensor("dvt", [128, TT, ET, 129], f16, kind="ExternalOutput")
        doT = nc.dram_tensor("doT", [128, ET, 512], f16, kind="ExternalOutput")

    Exp = mybir.ActivationFunctionType.Exp
    mult = mybir.AluOpType.mult
    add = mybir.AluOpType.add
    sub = mybir.AluOpType.subtract
    divide = mybir.AluOpType.divide

    with TileContext(nc) as tc:
        with (
            nc.allow_low_precision(reason="fp16 operands; fp32 PSUM accum"),
            tc.tile_pool(name="res", bufs=1) as res,
            tc.tile_pool(name="wpool", bufs=1) as wpool,
            tc.tile_pool(name="xpool", bufs=2) as xpool,
            tc.tile_pool(name="stgp", bufs=2) as stgp,
            tc.tile_pool(name="stgq", bufs=1) as stgq,
            tc.tile_pool(name="tp", bufs=2) as tp,
            tc.tile_pool(name="ptp", bufs=9) as ptp,
            tc.tile_pool(name="anp", bufs=4) as anp,
            tc.tile_pool(name="smp", bufs=6) as smp,
            tc.tile_pool(name="oTp", bufs=4) as oTp,
            tc.tile_pool(name="ostp", bufs=2) as ostp,
        ):
            # ---- resident tiles ----
            id_sb = res.tile([128, 128], f16, tag="id")
            m_sb = res.tile([128, ET, 512], f16, tag="masks")
            woh_sb = res.tile([128, ET, D], f8, tag="woh")
            wol_sb = res.tile([128, ET, D], f8, tag="wol")
            # Q ping-pong: chunk c's Q is written during sdpa(c-1) and only
            # read during sdpa(c), so two chunk-sized tiles suffice
            qt_t = [res.tile([128, ET, 512], f16, tag=f"qt{i}",
                             name=f"qt{i}") for i in range(2)]
            kt_sb = res.tile([128, ET, S], f16, tag="kt")
            vt_sb = res.tile([128, TT, ET, 129], f16, tag="vt")

            wqh_sb = wpool.tile([128, KT, ET, 128], f8, tag="wqh")
            wql_sb = wpool.tile([128, KT, ET, 128], f8, tag="wql")
            wkh_sb = wpool.tile([128, KT, ET, 128], f8, tag="wkh")
            wkl_sb = wpool.tile([128, KT, ET, 128], f8, tag="wkl")
            wvh_sb = wpool.tile([128, KT, E], f8, tag="wvh")
            wvl_sb = wpool.tile([128, KT, E], f8, tag="wvl")

            # ones column for the PV row-sum trick (scaled: see ASC)
            nc.vector.memset(vt_sb[:, :, :, 128:129], 1.0 / ASC)

            # full trig tables resident (2 DMAs total; HWDGE fixed cost
            # ~625ns/DMA makes per-chunk reloads a net loss)
            cos_sb = res.tile([128, S], f16, tag="cos")
            sin_sb = res.tile([128, S], f16, tag="sin")


            # ---- DMA loads; chunk 0 pairwise k-interleaved so the k-outer
            # projection streams at DMA pace ----
            # hi planes of wq/x first (they alone gate the main-product
            # sweep); lo planes + later-pass weights follow
            xc_t = {}
            xc0h = xpool.tile([128, KT, 512], f8, tag="xch")
            xc0l = xpool.tile([128, KT, 512], f8, tag="xcl")
            xc_t[0] = (xc0h, xc0l)
            # DMA priority order == chunk-0 sweep consumption order:
            # Q/K hi mains first, then lo correction planes, then V, then
            # everything sdpa(0)+ needs, then wo (chunk-1 time)
            for hf in range(4):
                ks = slice(hf * 4, (hf + 1) * 4)
                nc.sync.dma_start(wqh_sb[:, ks], wqTh[:, ks])
                nc.sync.dma_start(xc0h[:, ks], xTh[:, ks, 0:512])
            for hf in range(2):
                ks = slice(hf * 8, (hf + 1) * 8)
                nc.sync.dma_start(wkh_sb[:, ks], wkTh[:, ks])
            for hf in range(2):
                ks = slice(hf * 8, (hf + 1) * 8)
                nc.sync.dma_start(xc0l[:, ks], xTl[:, ks, 0:512])
            nc.sync.dma_start(wql_sb[:], wqTl[:])
            nc.sync.dma_start(cos_sb[:, 0:512], cosF[:, 0:512])
            nc.sync.dma_start(sin_sb[:, 0:512], sinF[:, 0:512])
            nc.sync.dma_start(wkl_sb[:], wkTl[:])
            nc.sync.dma_start(wvh_sb[:], wvTh[:])
            nc.sync.dma_start(wvl_sb[:], wvTl[:])

            def load_late():
                # emitted after the kouter so the rope-swap DMAs (emitted
                # inside it) sit AHEAD of these in the SP HWDGE FIFO
                nc.sync.dma_start(id_sb[:], ident[:])
                nc.sync.dma_start(m_sb[:], masks[:])
                load_xc(1)
                nc.sync.dma_start(cos_sb[:, 512:], cosF[:, 512:])
                nc.sync.dma_start(sin_sb[:, 512:], sinF[:, 512:])
                nc.sync.dma_start(woh_sb[:], woTh[:])
                nc.sync.dma_start(wol_sb[:], woTl[:])

            def load_xc(tcb):
                th = xpool.tile([128, KT, 512], f8, tag="xch")
                tl = xpool.tile([128, KT, 512], f8, tag="xcl")
                xc_t[tcb] = (th, tl)
                ts = slice(tcb * 512, (tcb + 1) * 512)
                nc.sync.dma_start(th[:], xTh[:, :, ts])
                nc.sync.dma_start(tl[:], xTl[:, :, ts])

            # Staged RoPE: each e-tile of a Q/K pass evacuates into a
            # contiguous fp16 staging tile; ONE pair of partition-strided
            # SBUF->SBUF DMAs then does the channel pair-swap for the whole
            # pass (replacing 4 PE permutation matmuls), and the cos/sin
            # combine runs all-fp16 on the DVE.
            def stage_evac(stag, pq, et):
                nc.scalar.copy(stag[:, et, :], pq[:])

            def rope_combine(stag, dst, dts, tcb, name):
                ts = slice(tcb * 512, (tcb + 1) * 512)
                c_t = cos_sb[:, ts]
                s_t = sin_sb[:, ts]
                stagP = stgq.tile([128, ET, 512], f16, tag="stagP",
                                  name=f"sp{name}")
                nc.sync.dma_start(stagP[0::2], stag[1::2])
                nc.sync.dma_start(stagP[1::2], stag[0::2])
                for et in range(ET):
                    t1 = tp.tile([128, 512], f16, tag="t1")
                    nc.vector.tensor_tensor(t1[:], stag[:, et, :], c_t,
                                            mult)
                    t2 = tp.tile([128, 512], f16, tag="t2")
                    nc.vector.tensor_tensor(t2[:], stagP[:, et, :], s_t,
                                            mult)
                    nc.vector.tensor_tensor(dst[:, et, dts], t1[:], t2[:],
                                            add)

            # ---- chunk-0 projection, k-outer with 4 concurrent
            # accumulators (own 5-bank scratch pool, closed before the
            # steady-state pools open) ----
            with tc.tile_pool(name="ps0", bufs=8, space="PSUM") as ps0:
                KP = KT // 2   # DoubleRow k-pairs

                def sweep0(accs, w_t, x_t, first=False, last=False):
                    for kp in range(KP):
                        for e in range(4):
                            nc.tensor.matmul(
                                accs[e][:],
                                w_t[:, 2 * kp:2 * kp + 2, e, :],
                                x_t[:, 2 * kp:2 * kp + 2, :],
                                start=(first and kp == 0),
                                stop=(last and kp == KP - 1),
                                perf_mode=DR)

                # Q and K main sweeps (hi planes only -- the first DMAs to
                # land) run before any correction sweep; corrections follow
                # in DMA arrival order.  Q+K accumulators fill all 8 banks.
                qaccs = [ps0.tile([128, 512], f32, tag="acc",
                                  name=f"p0q{i}") for i in range(4)]
                kaccs = [ps0.tile([128, 512], f32, tag="acc",
                                  name=f"p0k{i}") for i in range(4)]
                sweep0(qaccs, wqh_sb, xc0h, first=True)
                sweep0(kaccs, wkh_sb, xc0h, first=True)
                sweep0(qaccs, wqh_sb, xc0l)
                sweep0(qaccs, wql_sb, xc0h, last=True)
                stq = stgp.tile([128, ET, 512], f16, tag="stag", name="stq0")
                for et in range(ET):
                    stage_evac(stq, qaccs[et], et)
                rope_combine(stq, qt_t[0], slice(0, 512), 0, "q0")
                sweep0(kaccs, wkh_sb, xc0l)
                sweep0(kaccs, wkl_sb, xc0h, last=True)
                stk = stgp.tile([128, ET, 512], f16, tag="stag", name="stk0")
                for et in range(ET):
                    stage_evac(stk, kaccs[et], et)
                rope_combine(stk, kt_sb, slice(0, 512), 0, "k0")
                vaccs = [ps0.tile([128, 512], f32, tag="acc",
                                  name=f"p0v{i}") for i in range(4)]
                for si, (x_t, w_t) in enumerate(
                        ((xc0h, wvh_sb), (xc0l, wvh_sb))):
                    for kp in range(KP):
                        for tt in range(4):
                            nc.tensor.matmul(
                                vaccs[tt][:],
                                x_t[:, 2 * kp:2 * kp + 2,
                                    tt * 128:(tt + 1) * 128],
                                w_t[:, 2 * kp:2 * kp + 2, :],
                                start=(si == 0 and kp == 0), stop=False,
                                perf_mode=DR)
                # final sweep acc-outer: each vacc stops early so its
                # evacuation overlaps the remaining accs' matmuls
                for tt in range(4):
                    for kp in range(KP):
                        nc.tensor.matmul(
                            vaccs[tt][:],
                            xc0h[:, 2 * kp:2 * kp + 2,
                                 tt * 128:(tt + 1) * 128],
                            wvl_sb[:, 2 * kp:2 * kp + 2, :],
                            start=False, stop=(kp == KP - 1),
                            perf_mode=DR)
                    # early tiles on Act, late on DVE: Act must be clear
                    # when sdpa(0)'s first exp arrives right after
                    if tt < 2:
                        nc.scalar.mul(vt_sb[:, tt, :, 0:128], vaccs[tt][:],
                                      1.0 / WSC)
                    else:
                        nc.vector.tensor_scalar_mul(
                            vt_sb[:, tt, :, 0:128], vaccs[tt][:], 1.0 / WSC)
                load_late()

            # ---- steady state: one fused stream.  SDPA chunk c interleaved
            # with projection of chunk c+1 and output projection of chunk
            # c-1, which share a single 3-bank PSUM rotation ----
            with (
                tc.tile_pool(name="scp", bufs=1, space="PSUM") as scp,
                tc.tile_pool(name="pvp", bufs=1, space="PSUM") as pvp,
                tc.tile_pool(name="psA", bufs=2, space="PSUM") as psA,
            ):
                # two 2-bank slot tiles (not one 4-bank tile): identical
                # sdpa behavior, but the kernel tail can reclaim each slot
                # independently for the final wo accumulator ring
                sc_t = [scp.tile([128, 2, 512], f32, tag=f"sc{i}",
                                 name=f"sc{i}") for i in range(2)]
                # one full bank per concurrently-accumulating PV group: a
                # start=True matmul zeroes its ENTIRE 2KB bank (pending-zero),
                # so groups must never share a bank
                pv_ab = [pvp.tile([128, 256], f32, tag=f"pv{i}",
                                  name=f"pv{i}") for i in range(2)]

                KP = KT // 2

                def proj_units(tcb):
                    """Generator of filler units projecting chunk tcb."""
                    xch, xcl = xc_t[tcb]
                    if tcb + 1 < TC:
                        load_xc(tcb + 1)
                    for wi, (wh_sb, wl_sb, dst, dts) in enumerate(
                            ((wqh_sb, wql_sb, qt_t[tcb % 2], slice(0, 512)),
                             (wkh_sb, wkl_sb, kt_sb,
                              slice(tcb * 512, (tcb + 1) * 512)))):
                        stag = stgp.tile([128, ET, 512], f16, tag="stag",
                                         name=f"st{tcb}{wi}")
                        for et in range(ET):
                            pq = psA.tile([128, 512], f32, tag="acc")
                            n = 0
                            for w_t, x_t in ((wh_sb, xch), (wh_sb, xcl),
                                             (wl_sb, xch)):
                                for kp in range(KP):
                                    nc.tensor.matmul(
                                        pq[:],
                                        w_t[:, 2 * kp:2 * kp + 2, et, :],
                                        x_t[:, 2 * kp:2 * kp + 2, :],
                                        start=(n == 0), stop=(n == 23),
                                        perf_mode=DR)
                                    n += 1
                                    if n % 6 == 0:
                                        yield
                            stage_evac(stag, pq, et)
                            yield
                        rope_combine(stag, dst, dts, tcb, f"{tcb}{wi}")
                        yield
                    for tt in range(4):
                        pv = psA.tile([128, 512], f32, tag="acc")
                        n = 0
                        for x_t, w_t in ((xch, wvh_sb), (xcl, wvh_sb),
                                         (xch, wvl_sb)):
                            for kp in range(KP):
                                nc.tensor.matmul(
                                    pv[:],
                                    x_t[:, 2 * kp:2 * kp + 2,
                                        tt * 128:(tt + 1) * 128],
                                    w_t[:, 2 * kp:2 * kp + 2, :],
                                    start=(n == 0), stop=(n == 23),
                                    perf_mode=DR)
                                n += 1
                                if n % 6 == 0:
                                    yield
                        nc.scalar.mul(
                            vt_sb[:, tcb * 4 + tt, :, 0:128], pv[:],
                            1.0 / WSC)
                        yield

                ost_of = {}
                wo_ctr = [0]
                OSC = 1.0 / (ASC * WSC)

                def wo_units(oT_c, ic, act_mod=2, final=False,
                             tls=(0, 1, 2, 3), po_ring=None):
                    """Generator of filler units: output projection of
                    chunk ic ((tl,dc) groups, fp8 DoubleRow 3-term)."""
                    oTh_c, oTl_c = oT_c
                    gi = 0
                    for tl in tls:
                        tsl = slice(tl * 128, (tl + 1) * 128)
                        for dc in range(4):
                            g = wo_ctr[0]
                            wo_ctr[0] += 1
                            dsl = slice(dc * 512, (dc + 1) * 512)
                            if po_ring is not None:
                                po = po_ring[gi % len(po_ring)]
                                gi += 1
                            else:
                                po = psA.tile([128, 512], f32, tag="acc")
                            n = 0
                            for a_t, w_t in ((oTh_c, woh_sb), (oTl_c, woh_sb),
                                             (oTh_c, wol_sb)):
                                for ep in range(ET // 2):
                                    nc.tensor.matmul(
                                        po[:],
                                        a_t[:, 2 * ep:2 * ep + 2, tsl],
                                        w_t[:, 2 * ep:2 * ep + 2, dsl],
                                        start=(n == 0), stop=(n == 5),
                                        perf_mode=DR)
                                    n += 1
                            key = (id(oTh_c), tl)
                            if key not in ost_of:
                                ost_t = ostp.tile([128, D], f16, tag="ost",
                                                  name=f"ost{ic}_{tl}")
                                ost_of[key] = (ost_t, 4 * ic + tl)
                            ost, ttk = ost_of[key]
                            if g % act_mod == 0:
                                nc.scalar.mul(ost[:, dsl], po[:], OSC)
                            else:
                                nc.vector.tensor_scalar_mul(
                                    ost[:, dsl], po[:], OSC)
                            if final and tl == 3:
                                # kernel tail: per-dc DMAs pipeline the final
                                # writes with the copies (HWDGE is idle here)
                                nc.sync.dma_start(
                                    out[ttk * 128:(ttk + 1) * 128, dsl],
                                    ost[:, dsl])
                            elif dc == 3:
                                # one whole-row DMA per 128-token tile: the
                                # HWDGE fixed cost dwarfs the extra transfer
                                nc.sync.dma_start(
                                    out[ttk * 128:(ttk + 1) * 128, :],
                                    ost[:])
                            yield

                def chain(*gens):
                    for g in gens:
                        yield from g

                evac_pending = []

                def evac_front(job):
                    # rowsum col is (sum p)/ASC, so the normalize multiply
                    # yields attn*ASC -- a good fp8 range for the wo matmul
                    pvx, _oT, _h, _it = job
                    sm = smp.tile([128, 1], f32, tag="sm")
                    nc.vector.reciprocal(sm[:], pvx[:, 128:129])
                    an = anp.tile([128, 128], f16, tag="an")
                    nc.vector.tensor_scalar_mul(
                        an[:], pvx[:, 0:128], sm[:])
                    return an

                def evac_back(an, job):
                    pvx, (oTh_c, oTl_c), h_, it = job
                    tpv = pvx.bitcast(f16)[:, 280:408]
                    nc.tensor.transpose(tpv, an[:], id_sb[:])
                    osl = slice(it * 128, (it + 1) * 128)
                    nc.vector.tensor_copy(oTh_c[:, h_, osl], tpv)
                    nc.vector.tensor_tensor(
                        oTl_c[:, h_, osl], tpv, oTh_c[:, h_, osl], sub)

                def emit_evac(job):
                    evac_back(evac_front(job), job)

                def sdpa_chunk(ic, filler, n_fill_units, primed=False,
                               prime_ic=None):
                    nj = 4 * (ic + 1)
                    npair = nj // 2
                    qt_c = qt_t[ic % 2]
                    oTh_ic = oTp.tile([128, ET, 512], f8, tag="oTh",
                                      name=f"oTh{ic}")
                    oTl_ic = oTp.tile([128, ET, 512], f8, tag="oTl",
                                      name=f"oTl{ic}")
                    oT_ic = (oTh_ic, oTl_ic)
                    total_steps = ET * (2 * npair + 2)
                    state = {"step": 0, "filled": 0}

                    def fill(n_steps=1):
                        state["step"] += n_steps
                        want = (state["step"] * n_fill_units) // total_steps
                        while state["filled"] < want:
                            try:
                                next(filler)
                            except StopIteration:
                                break
                            state["filled"] += 1

                    for h in range(ET):
                        # diagonal pairs last: the scores-ahead pipeline hides
                        # their exp->mask chain behind earlier pairs' work
                        pairs = (list(range(0, 2 * ic))
                                 + list(range(2 * ic, npair)))

                        def emit_scores(idx, hh=None):
                            if hh is None:
                                hh = h
                            p = pairs[idx]
                            slot = idx % 2
                            for half in range(2):
                                jt = 2 * p + half
                                # causal: i-columns below the diagonal tile
                                # are fully masked; start=True bank-zeroes
                                # them, exp(0)=1 is killed by the mask mult
                                lo = max(0, (jt - 4 * ic) * 128)
                                nc.tensor.matmul(
                                    sc_t[slot][:, half, lo:],
                                    kt_sb[:, hh, jt * 128:(jt + 1) * 128],
                                    qt_c[:, hh, lo:], start=True, stop=True)

                        def emit_pv(its, idx, pt_x):
                            p = pairs[idx]
                            for half in range(2):
                                jt = 2 * p + half
                                st = (idx == 0 and half == 0)
                                for sl, it in enumerate(its):
                                    git = 4 * ic + it
                                    if jt > git:
                                        # fully-masked tile: contributes 0
                                        continue
                                    nc.tensor.matmul(
                                        pv_ab[sl][:, 0:129],
                                        pt_x[:, half,
                                             it * 128:(it + 1) * 128],
                                        vt_sb[:, jt, h, :],
                                        start=st, stop=(jt == git))

                        if h == 0 and not primed:
                            emit_scores(0)
                            if npair > 1:
                                emit_scores(1)
                            # no prior-chunk prime covered this exp chain:
                            # release extra filler behind the first scores
                            fill(3)
                        # (h>0: previous head's stage-b primed our scores)
                        # the previous head's deferred it2/3 evacs read the
                        # same PV slots stage-a is about to overwrite - they
                        # MUST all be emitted before the first PV below
                        while evac_pending:
                            evac_back(*evac_pending.pop(0))
                        pts = []
                        # stage a: exp + PV of i-tiles 0/1
                        for idx in range(npair):
                            pt_x = ptp.tile([128, 2, 512], f16, tag="pt")
                            pts.append(pt_x)
                            nc.scalar.activation(
                                pt_x[:],
                                sc_t[idx % 2][:],
                                Exp, scale=SCALE)
                            m = 2 * pairs[idx] - 4 * ic
                            if m >= 0:
                                nc.vector.tensor_tensor(
                                    pt_x[:], pt_x[:], m_sb[:, m:m + 2, :],
                                    mult)
                            if idx + 2 < npair:
                                emit_scores(idx + 2)
                            fill()
                            emit_pv((0, 1), idx, pt_x)
                        emit_evac((pv_ab[0], oT_ic, h, 0))
                        emit_evac((pv_ab[1], oT_ic, h, 1))
                        fill()
                        # stage b: PV of i-tiles 2/3 off the saved pts -
                        # exp-free PE work that covers the evac chains
                        for idx in range(npair):
                            emit_pv((2, 3), idx, pts[idx])
                            if h + 1 < ET:
                                if idx == 0:
                                    emit_scores(0, h + 1)
                                if idx == min(1, npair - 1) and npair > 1:
                                    emit_scores(1, h + 1)
                            fill()
                        fill()
                        # run the DVE front half (recip+normalize) now; only
                        # the transpose+copies wait for the next drain point,
                        # so the boundary transpose finds `an` ready
                        j2 = (pv_ab[0], oT_ic, h, 2)
                        j3 = (pv_ab[1], oT_ic, h, 3)
                        evac_pending.append((evac_front(j2), j2))
                        evac_pending.append((evac_front(j3), j3))
                    if prime_ic is None:
                        # last chunk: nothing downstream hides the deferred
                        # evacs -- emit them now so they overlap the drain
                        while evac_pending:
                            evac_back(*evac_pending.pop(0))
                    # drain any unconsumed filler at chunk end (before the
                    # prime: prime matmuls queued ahead of leftover filler
                    # would stall PE on the next chunk's rope chain)
                    while True:
                        try:
                            next(filler)
                        except StopIteration:
                            break
                    # prime the NEXT chunk's first two score pairs
                    if prime_ic is not None:
                        qt_n = qt_t[prime_ic % 2]
                        for jt in range(4):
                            nc.tensor.matmul(
                                sc_t[jt // 2][:, jt % 2, :],
                                kt_sb[:, 0, jt * 128:(jt + 1) * 128],
                                qt_n[:, 0, :], start=True, stop=True)
                    return oT_ic

                oT_hist = {}
                oT_hist[0] = sdpa_chunk(0, proj_units(1), 62, prime_ic=1)
                if DEBUG_DUMPS:
                    nc.sync.dma_start(dkt[:], kt_sb[:])
                    nc.sync.dma_start(dvt[:], vt_sb[:])
                    nc.sync.dma_start(doT[:], oT_hist[0][0][:])
                oT_hist[1] = sdpa_chunk(
                    1, proj_units(2), 66, primed=True, prime_ic=2)
                oT_hist[2] = sdpa_chunk(2, proj_units(3), 62,
                                        primed=True, prime_ic=3)
                # all deferred wo work lands in chunk 3: it has the largest
                # sdpa latency chains (8 pairs/head) and no proj filler left
                oT_hist[3] = sdpa_chunk(
                    3, chain(wo_units(oT_hist[0], 0, act_mod=3),
                             wo_units(oT_hist[1], 1, act_mod=10 ** 9),
                             wo_units(oT_hist[2], 2, act_mod=4)), 44,
                    primed=True)
                while evac_pending:
                    evac_back(*evac_pending.pop(0))
                # 3-deep accumulator rotation for the serial tail: two psA
                # banks plus ONE reclaimed score bank (single slice -> one
                # group at a time on that tile, no intra-tile group serial)
                poA = scp.tile([128, 2, 512], f32, tag="sc0", name="wo_scA")
                poB = scp.tile([128, 2, 512], f32, tag="sc1", name="wo_scB")
                ring = [psA.tile([128, 512], f32, tag="acc", name="wo_pa0"),
                        psA.tile([128, 512], f32, tag="acc", name="wo_pa1"),
                        poA[:, 0, :], poB[:, 0, :]]
                for _ in wo_units(oT_hist[3], 3, act_mod=1, final=True,
                                  po_ring=ring):
                    pass

    _split_waits(nc)
    return nc


_NC = None


def _get_nc():
    global _NC
    if _NC is None:
        _NC = _build_nc()
    return _NC


# ---------------------------------------------------------------------------
# Host-side prep + gather
# ---------------------------------------------------------------------------

def _rope_tables():
    # pre-scaled by 1/WSC: the rope combine folds the 2**6 fp8 weight
    # pre-scale back out of the Q/K projection PSUM for free
    j = np.arange(0, HD, 2, dtype=np.float32) / HD
    inv_freq = (1.0 / (ROPE_BASE ** j)).astype(np.float32)           # [64]
    t = np.arange(S, dtype=np.float32)
    freqs = np.outer(t, inv_freq)                                    # [S, 64]
    cos = np.cos(freqs).astype(np.float32) / WSC                     # [S, 64]
    sin = np.sin(freqs).astype(np.float32) / WSC
    cosF = np.empty((128, S), dtype=np.float32)
    sinF = np.empty((128, S), dtype=np.float32)
    cosF[0::2, :] = cos.T
    cosF[1::2, :] = cos.T
    sinF[0::2, :] = -sin.T
    sinF[1::2, :] = sin.T
    return cosF.astype(np.float16), sinF.astype(np.float16)


def _static_inputs():
    cosF, sinF = _rope_tables()
    ident = np.eye(128, dtype=np.float16)
    masks = np.zeros((128, ET, 512), dtype=np.float16)
    il = np.arange(512)
    for m in range(ET):
        for p in range(128):
            masks[p, m, :] = (il >= 128 * m + p).astype(np.float16)
    return {
        "cosF": cosF, "sinF": sinF,
        "ident": ident, "masks": masks,
    }


def _fp8_split(a):
    """a (float32) -> (hi, lo) float8_e4m3 with hi + lo ~= a."""
    hi = a.astype(F8)
    lo = (a - hi.astype(np.float32)).astype(F8)
    return hi, lo


def _core_inputs(x, wqk, wv, wo, static, b, g):
    xb = np.ascontiguousarray(x[b].T)                                # [D, S]
    xT = np.ascontiguousarray(
        xb.reshape(KT, 128, S).transpose(1, 0, 2)).astype(np.float32)
    xTh, xTl = _fp8_split(xT)

    wq_g = wqk[E * g:E * (g + 1), :]                                 # [E, D]
    wk_g = wqk[D + E * g:D + E * (g + 1), :]
    wv_g = wv[E * g:E * (g + 1), :]
    wqT = np.ascontiguousarray(
        wq_g.T.reshape(KT, 128, ET, 128)
        .transpose(1, 0, 2, 3)).astype(np.float32) * WSC
    wkT = np.ascontiguousarray(
        wk_g.T.reshape(KT, 128, ET, 128)
        .transpose(1, 0, 2, 3)).astype(np.float32) * WSC
    wvT = np.ascontiguousarray(
        wv_g.T.reshape(KT, 128, E).transpose(1, 0, 2)).astype(np.float32) * WSC
    wqTh, wqTl = _fp8_split(wqT)
    wkTh, wkTl = _fp8_split(wkT)
    wvTh, wvTl = _fp8_split(wvT)
    woT = np.ascontiguousarray(
        wo[:, E * g:E * (g + 1)].T.reshape(ET, 128, D)
        .transpose(1, 0, 2)).astype(np.float32) * WSC
    woTh, woTl = _fp8_split(woT)

    m = dict(static)
    m.update({"xTh": xTh, "xTl": xTl,
              "wqTh": wqTh, "wqTl": wqTl,
              "wkTh": wkTh, "wkTl": wkTl,
              "wvTh": wvTh, "wvTl": wvTl,
              "woTh": woTh, "woTl": woTl})
    return m


def kernel(x, wqk, wv, wo):
    x = np.asarray(x, dtype=np.float32)
    wqk = np.asarray(wqk, dtype=np.float32)
    wv = np.asarray(wv, dtype=np.float32)
    wo = np.asarray(wo, dtype=np.float32)

    nc = _get_nc()
    static = _static_inputs()
    in_maps = [
        _core_inputs(x, wqk, wv, wo, static, c // G, c % G) for c in range(8)
    ]
    res = run_bass_kernel_spmd(nc, in_maps, core_ids=list(range(8)))
    out = np.zeros((B, S, D), dtype=np.float32)
    for c in range(8):
        out[c // G] += res.results[c]["out"].astype(np.float32)
    return out

